# revision 60
# baseline (speedup 1.0000x reference)
"""Causal multi-head attention with RoPE on 8 Trainium2 NeuronCores (Bass/Tile).

Problem: B=2, S=2048, E=768, H=12 heads, D=64, full rotary (ROPE_DIM=D),
causal softmax, fused QKV + output projection.

Sharding: 8 cores = 2 batches x 4 head-groups (3 heads each).

v2 (bf16, ~146-153us vs the 207us fp32r baseline):
  - all matmul inputs bf16 (PSUM accumulation stays fp32); y partials
    written back as bf16 and upcast on the host,
  - q&k projections fused into 3 full 128-partition chunks (2 head-dim
    blocks each: [q0|q1], [k0|k1], [q2|k2]) so neither the PE array nor
    the RoPE vector ops ever run at M=64; q2 re-copied to partition
    rows 64:128 (4x-mode DVE copy) so head 2's score matmul sees q and
    k at the same base partition,
  - q/k bias folded into the ScalarE PSUM->SBUF copy (per-partition
    bias operand) so RoPE is two 2x-mode bf16 DVE multiplies + one add,
  - attention per (query-block, head) unit with diagonal key-blocks
    first and the AV matmuls software-pipelined 2 score-tiles behind,
    so PE's in-order stream never blocks on the exp -> causal-mask
    chain (ScalarE exp -> gpsimd affine_select -> AV),
  - softmax denominator 1/den = exp(-ln den) on ScalarE (same
    activation table as the score exps -> no table reloads), partition-
    broadcast via a DRAM-roundtrip stride-0 DMA, multiply on DVE; the
    whole chain is emitted inside the NEXT unit's kp loop and each
    query block's y projection (bf16 half-row pieces) inside the NEXT
    block's heads 1-2 at kp>=2, so in-order engines never head-of-line
    block on them,
  - x streamed in eighth-column chunks and wqk split per chunk so the
    first projection matmul starts ~4us in.
Rejected on HW measurement: K=1 ones-matmul broadcast of 1/den
(PE slow path), DVE InstReciprocal (~5x slower than modeled), moving
the half-1 projections into the attention phase (pt/s2 pool
contention), gpsimd partition_broadcast + custom-DVE ops (this
walrus can't codegen InstISA), MAX_WAITS>1 (walrus cap).
Host sums the 4 partials per batch and adds (bp + bv @ Wp) once.
"""
import math

import numpy as np

N_HEADS = 12
ROPE_BASE = 10000.0
B, S, E = 2, 2048, 768
D = 64
HPC = 3            # heads per core
N_CORES = 8
QB = 512           # query block (free dim of score tiles)
KB = 128           # key block (partition dim of score tiles)
NQB = S // QB      # 4
NKB = S // KB      # 16
EK = E // 128      # 6 contraction chunks

_RUNNER = None


# ---------------------------------------------------------------- tile patch
def _patch_tile_drain():
    """This container's walrus caps semaphore waits per instruction ("Too
    many sync wait commands").  Split the TileContext tail-drain waits
    across dedicated SP nops."""
    import concourse.tile as tile
    import concourse.mybir as mybir

    if getattr(tile.TileContext, "_drain_patched", False):
        return

    def _drain_and_barrier(self, tick_clock, wait_clock):
        nc = self.nc
        drain_inst = nc.sync.drain()
        wait_clock.add_sem_waits(
            drain_inst.ins, tile.ScopedClock({None: tick_clock.global_clock})
        )
        si = drain_inst.ins.sync_info
        waits = list(si.on_wait) if si is not None else []
        if len(waits) > 1:
            drain_inst.ins.sync_info.on_wait = waits[:1]
            for w in waits[1:]:
                n = nc.sync.nop(nofuse=True)
                n.ins.sync_info = mybir.SyncInfo(on_wait=[w], on_update=[])
        nc.all_engine_barrier()
        assert self.sems is not None
        popped = nc._tile_sem_poison_stack.pop()
        assert popped is self._sem_poison
        nc.clear_and_free_semaphores(list(self.sems.allocated().values()))
        nc.all_engine_barrier()

    tile.TileContext._drain_and_barrier = _drain_and_barrier
    tile.TileContext._drain_patched = True


MAX_WAITS = 1


def _split_waits(nc, maxw=None):
    """Move excess semaphore waits onto same-engine NoOps inserted just
    before the carrying instruction (walrus per-instruction wait cap)."""
    import concourse.mybir as mybir

    if maxw is None:
        maxw = MAX_WAITS
    k = 0
    for f in nc.m.functions:
        for bb in f.blocks:
            new = []
            for ins in bb.instructions:
                si = ins.sync_info
                if si is not None and len(si.on_wait) > maxw:
                    waits = list(si.on_wait)
                    head, tail = waits[:-maxw], waits[-maxw:]
                    for i in range(0, len(head), maxw):
                        nop = mybir.InstNoOp(
                            name=f"{ins.name}-sw{k}", ins=[], outs=[])
                        k += 1
                        nop.engine = ins.engine
                        nop.sync_info = mybir.SyncInfo(
                            on_wait=head[i:i + maxw], on_update=[])
                        new.append(nop)
                    si.on_wait = tail
                new.append(ins)
            bb.instructions = new


# ---------------------------------------------------------------- device IR
def build_bass(reps=1):
    """reps>1 wraps the whole kernel in an on-device For_i repeat loop --
    used only for timing (slope between rep counts removes dispatch
    overhead)."""
    import contextlib
    import concourse.bass as bass
    import concourse.mybir as mybir
    import concourse.tile as tile

    _patch_tile_drain()
    f32 = mybir.dt.float32
    bf16 = mybir.dt.bfloat16
    Act = mybir.ActivationFunctionType
    Alu = mybir.AluOpType

    nc = bass.Bass(enable_partition_id=False)
    xT = nc.dram_tensor("xT", [E, S], bf16, kind="ExternalInput")
    wqk = nc.dram_tensor("wqk", [E, 384], bf16, kind="ExternalInput")
    wv = nc.dram_tensor("wv", [E, 256], bf16, kind="ExternalInput")
    wp = nc.dram_tensor("wp", [HPC * D, E], bf16, kind="ExternalInput")
    biasesd = nc.dram_tensor("biases", [128, 3], f32, kind="ExternalInput")
    trigd = nc.dram_tensor("trig", [128, 2 * S], bf16, kind="ExternalInput")
    smallsd = nc.dram_tensor("smalls", [128, 2432], bf16, kind="ExternalInput")
    dend = nc.dram_tensor("dend", [12, 512], f32, kind="Internal")
    y = nc.dram_tensor("y", [S, E], bf16, kind="ExternalOutput")

    with tile.TileContext(nc) as tc:
        rep_loop = (
            tc.For_i(0, reps, 1,
                     hint_engines=(mybir.EngineType.PE, mybir.EngineType.DVE,
                                   mybir.EngineType.Activation,
                                   mybir.EngineType.Pool, mybir.EngineType.SP))
            if reps > 1 else contextlib.nullcontext()
        )
        with rep_loop, (
            tc.tile_pool(name="consts", bufs=1)
        ) as consts, tc.tile_pool(name="big", bufs=1) as big:
            # ---- constant loads.  wqk first (gates the first matmul),
            # then x in quarter-column chunks so compute starts early.
            wqk_all = consts.tile([128, EK * 384], bf16, tag="wqk_all")
            wqk_s = wqk_all.rearrange("p (a m) -> p a m", a=EK)
            wqk_d = wqk.rearrange("(a p) m -> p a m", p=128)
            nc.sync.dma_start(out=wqk_s[:, :, 0:128], in_=wqk_d[:, :, 0:128])
            wqk_t = [wqk_all[:, e * 384:(e + 1) * 384] for e in range(EK)]
            biases_t = consts.tile([128, 3], f32, tag="biases")
            nc.sync.dma_start(out=biases_t, in_=biasesd[:, :])
            xt_all = big.tile([128, EK * S], bf16, tag="xt_all")
            xt3 = xt_all.rearrange("p (a m) -> p a m", a=EK)
            xs3 = xT.rearrange("(a p) m -> p a m", p=128)

            def load_x(i8, n=1):
                nc.sync.dma_start(
                    out=xt3[:, :, i8 * 256:(i8 + n) * 256],
                    in_=xs3[:, :, i8 * 256:(i8 + n) * 256])

            load_x(0)
            load_x(1)
            nc.sync.dma_start(out=wqk_s[:, :, 128:384],
                              in_=wqk_d[:, :, 128:384])
            xt = [xt_all[:, e * S:(e + 1) * S] for e in range(EK)]
            smalls_t = consts.tile([128, 2432], bf16, tag="smalls")
            nc.sync.dma_start(out=smalls_t, in_=smallsd[:, :])
            p2_t = smalls_t[:, 0:128]
            ones_row = smalls_t[0:1, 128:128 + S]
            wv7 = smalls_t[0:1, 128 + S:128 + S + 256]
            wv_all = consts.tile([128, EK * 256], bf16, tag="wv_all")
            nc.sync.dma_start(
                out=wv_all.rearrange("p (a m) -> p a m", a=EK),
                in_=wv.rearrange("(a p) m -> p a m", p=128))
            wv_t = [wv_all[:, e * 256:(e + 1) * 256] for e in range(EK)]
            load_x(2, 2)
            trig_t = consts.tile([128, 2 * S], bf16, tag="trig")
            nc.sync.dma_start(out=trig_t, in_=trigd[:, :])
            cos_t = trig_t[:, 0:S]
            sin_t = trig_t[:, S:2 * S]
            load_x(4, 2)
            load_x(6, 2)
            wp0 = consts.tile([128, E], bf16, tag="wp0")
            nc.sync.dma_start(out=wp0, in_=wp[0:128, :])
            wp1 = consts.tile([64, E], bf16, tag="wp1")
            nc.sync.dma_start(out=wp1, in_=wp[128:192, :])

            # ---- long-lived activations: 3 fused q|k chunks, each 128
            # partitions = 2 head-dim blocks of 64.
            # chunk0 = q heads 0,1 ; chunk1 = k heads 0,1 ;
            # chunk2 = q head 2 | k head 2.  Scores need lhsT/rhs at the
            # same base partition, so q2 is re-copied to rows 64:128 of a
            # scratch tile (4x-mode DVE copy).
            qk_c = [big.tile([128, S], bf16, tag=f"qk{c}", name=f"qk{c}")
                    for c in range(3)]
            q2scr = big.tile([128, S], bf16, tag="q2scr", name="q2scr")
            v2_sb = [big.tile([128, 512], bf16, tag=f"v2_{s}", name=f"v2_{s}")
                     for s in range(NKB // 2)]
            oTa_q = [big.tile([128, QB], bf16, tag=f"oTa{qb}",
                              name=f"oTa{qb}") for qb in range(NQB)]
            oTb_q = [big.tile([64, QB], bf16, tag=f"oTb{qb}",
                              name=f"oTb{qb}") for qb in range(NQB)]

            # ============================ phase 1: projections + RoPE
            with (
                tc.tile_pool(name="psq", bufs=2, space="PSUM") as psq_pool,
                tc.tile_pool(name="psrot", bufs=1, space="PSUM") as rot_pool,
                tc.tile_pool(name="psv", bufs=2, space="PSUM") as psv_pool,
                tc.tile_pool(name="ropetmp", bufs=2) as rtmp,
            ):
                def emit_qk_chunk(ch, half, pools=None):
                    ps_pool, r_pool, t_pool, ps_tag, t_tag = pools or (
                        psq_pool, rot_pool, rtmp, "psq", None)
                    c0 = half * 1024
                    ps = ps_pool.tile([128, 1024], f32, tag=ps_tag)
                    for n in range(4):
                        for e in range(EK):
                            nc.tensor.matmul(
                                ps[:, n * 256:(n + 1) * 256],
                                lhsT=wqk_t[e][:, ch * 128:(ch + 1) * 128],
                                rhs=xt[e][:, c0 + n * 256:c0 + (n + 1) * 256],
                                start=(e == 0), stop=(e == EK - 1),
                            )
                    # biased q -> SBUF bf16 via ScalarE (bias per partition)
                    q_sb = t_pool.tile([128, 1024], bf16,
                                       tag=t_tag or "qsb")
                    nc.scalar.activation(q_sb, ps, Act.Identity,
                                         bias=biases_t[:, ch:ch + 1])
                    # qc = q_sb * cos   (all-bf16: 2x DVE mode)
                    qc = t_pool.tile([128, 1024], bf16, tag=t_tag or "qc")
                    nc.vector.tensor_mul(qc, q_sb, cos_t[:, c0:c0 + 1024])
                    # qrot = P2 @ q_sb ; qs = qrot * sin ; out = qc + qs
                    rot = r_pool.tile([128, 1024], f32,
                                      tag="rot" if pools is None else ps_tag)
                    for n in range(2):
                        nc.tensor.matmul(
                            rot[:, n * 512:(n + 1) * 512], lhsT=p2_t,
                            rhs=q_sb[:, n * 512:(n + 1) * 512],
                            start=True, stop=True)
                    qs = t_pool.tile([128, 1024], bf16, tag=t_tag or "qs")
                    nc.vector.tensor_mul(qs, rot, sin_t[:, c0:c0 + 1024])
                    nc.vector.tensor_add(qk_c[ch][:, c0:c0 + 1024], qc, qs)
                    if ch == 2:
                        nc.vector.tensor_copy(
                            q2scr[64:128, c0:c0 + 1024],
                            qk_c[2][0:64, c0:c0 + 1024])

                def emit_v_block(s0):
                    # two sblocks share one PSUM bank; one copy out
                    ps = psv_pool.tile([128, 512], f32, tag="psv")
                    for i in range(2):
                        s = s0 + i
                        c = i * 256
                        for e in range(EK):
                            nc.tensor.matmul(
                                ps[:, c:c + 256],
                                lhsT=xt[e][:, s * 128:(s + 1) * 128],
                                rhs=wv_t[e], start=(e == 0), stop=False)
                        nc.tensor.matmul(
                            ps[:, c:c + 256],
                            lhsT=ones_row[:, s * 128:(s + 1) * 128],
                            rhs=wv7, start=False, stop=True)
                    nc.vector.tensor_copy(v2_sb[s0 // 2], ps)

                # half 0 of all 3 chunks first so attention and the
                # second x half DMA overlap phase 1's tail.  v blocks for
                # keys 1024+ (first needed by qb2) are deferred into the
                # qb0/qb1 unit boundaries to shorten the serial PE span
                # before the first attention exp.
                chunks = [(ch, half) for half in range(2) for ch in range(3)]
                vs = iter(range(0, 8, 2))
                for ch, half in chunks:
                    emit_qk_chunk(ch, half)
                    s = next(vs, None)
                    if s is not None:
                        emit_v_block(s)
                for s in vs:
                    emit_v_block(s)
                emit_v_late = emit_v_block

            # ============================ phase 2+3: attention + y proj
            def v_lhsT(s, h):
                # head values cols [65h..65h+63] + ones col at 65h+64
                return v2_sb[s // 2][:, (s % 2) * 256 + 65 * h:
                                     (s % 2) * 256 + 65 * h + 65]

            # per-head (qT tile, kT tile, partition row for both)
            heads = ((qk_c[0], qk_c[1], 0),
                     (qk_c[0], qk_c[1], 64),
                     (q2scr, qk_c[2], 64))

            with (
                tc.tile_pool(name="ps_s", bufs=2, space="PSUM") as s_pool,
                tc.tile_pool(name="ps_ov", bufs=2, space="PSUM") as ov_pool,
                tc.tile_pool(name="ps_y", bufs=2, space="PSUM") as y_pool,
                tc.tile_pool(name="pt", bufs=6) as pt_pool,
                tc.tile_pool(name="eps", bufs=2) as ep_pool,
                tc.tile_pool(name="ysb", bufs=2) as ysb_pool,
            ):
                def emit_v_late2(s0):
                    ps = ov_pool.tile([128, 512], f32, tag="ov")
                    for i in range(2):
                        s = s0 + i
                        c = i * 256
                        for e in range(EK):
                            nc.tensor.matmul(
                                ps[:, c:c + 256],
                                lhsT=xt[e][:, s * 128:(s + 1) * 128],
                                rhs=wv_t[e], start=(e == 0), stop=False)
                        nc.tensor.matmul(
                            ps[:, c:c + 256],
                            lhsT=ones_row[:, s * 128:(s + 1) * 128],
                            rhs=wv7, start=False, stop=True)
                    nc.vector.tensor_copy(v2_sb[s0 // 2], ps)

                def emit_den(qb, h, ov):
                    # normalize, inside the NEXT unit: 1/den = exp(-ln den)
                    # on ScalarE (same activation table as the score exps),
                    # partition-broadcast via a DRAM roundtrip (stride-0
                    # read), multiply on DVE
                    import concourse.bass as _b
                    dl = ep_pool.tile([1, 512], f32, tag="dl")
                    nc.scalar.activation(dl, ov[64:65, :], Act.Ln)
                    rec_sb = ep_pool.tile([1, 512], f32, tag="den")
                    nc.scalar.activation(rec_sb, dl, Act.Exp, scale=-1.0)
                    ei = 3 * qb + h
                    nc.sync.dma_start(out=dend[ei:ei + 1, :], in_=rec_sb)
                    recb = ep_pool.tile([64, 512], f32, tag="denb")
                    dsrc = dend[ei:ei + 1, :]
                    nc.sync.dma_start(
                        out=recb,
                        in_=_b.AP(tensor=dsrc.tensor, offset=dsrc.offset,
                                  ap=[[0, 64]] + list(dsrc.ap[1:])))
                    if h < 2:
                        dst = oTa_q[qb][64 * h:64 * h + 64, :]
                    else:
                        dst = oTb_q[qb]
                    nc.vector.tensor_mul(dst, ov[0:64, :], recb)

                def emit_y(qb, mi, c0):
                    # one half-row-block piece: [128 queries, 384 cols]
                    m = 4 * qb + mi
                    yp = y_pool.tile([128, 384], f32, tag="yp")
                    nc.tensor.matmul(
                        yp,
                        lhsT=oTa_q[qb][:, mi * 128:(mi + 1) * 128],
                        rhs=wp0[:, c0:c0 + 384],
                        start=True, stop=False)
                    nc.tensor.matmul(
                        yp,
                        lhsT=oTb_q[qb][:, mi * 128:(mi + 1) * 128],
                        rhs=wp1[:, c0:c0 + 384],
                        start=False, stop=True)
                    y_sb = ysb_pool.tile([128, 384], bf16, tag="ysb")
                    nc.vector.tensor_copy(y_sb, yp)
                    nc.sync.dma_start(
                        out=y[m * 128:(m + 1) * 128, c0:c0 + 384], in_=y_sb)

                # Software pipeline: each head's den-chain (DVE/Pool only)
                # is emitted inside the NEXT head's kp loop, and each query
                # block's y projection inside the NEXT block's heads, so
                # in-order engines never head-of-line block on them.
                pend_den = None           # (qb, h, ov) awaiting den chain
                pend_y = []               # [(qb, mi)] awaiting y projection
                pend_v = [8, 10, 12, 14]  # deferred v blocks (keys 1024+)
                for qb in range(NQB):
                    for h, (qt, kt, pr) in enumerate(heads):
                        if 1 <= 3 * qb + h <= 4 and pend_v:
                            emit_v_late2(pend_v.pop(0))
                        ov = ov_pool.tile([128, 512], f32, tag="ov")
                        qslice = qt[pr:pr + 64, qb * 512:(qb + 1) * 512]
                        nkb = 4 * (qb + 1)
                        # diagonal blocks first: their mask latency hides
                        # behind the past-key matmuls that follow
                        kbs = list(range(4 * qb, nkb)) + list(range(4 * qb))
                        # y half-pieces of the previous qb go into heads
                        # 1-2 at kp>=2, far enough from the den DMA
                        # roundtrip they depend on
                        y_slots = list(range(2, nkb // 2)) if h >= 1 else []
                        def emit_av(kp, pair, pt2):
                            for j, kb in enumerate(pair):
                                nc.tensor.matmul(
                                    ov[0:65, :], lhsT=v_lhsT(kb, h),
                                    rhs=pt2[:, j * 512:(j + 1) * 512],
                                    start=(2 * kp + j == 0),
                                    stop=(2 * kp + j == nkb - 1))

                        # AV runs 2 kp-steps behind scores so PE's in-order
                        # stream never blocks on the exp -> mask chain
                        inflight = []
                        for kp in range(nkb // 2):
                            pair = kbs[2 * kp:2 * kp + 2]
                            s2 = s_pool.tile([128, 1024], f32, tag="s2")
                            pt2 = pt_pool.tile([128, 1024], bf16, tag="pt2")
                            for j, kb in enumerate(pair):
                                nc.tensor.matmul(
                                    s2[:, j * 512:(j + 1) * 512],
                                    lhsT=kt[pr:pr + 64,
                                            kb * 128:(kb + 1) * 128],
                                    rhs=qslice, start=True, stop=True)
                            nc.scalar.activation(
                                pt2, s2, Act.Exp, scale=1.0 / math.sqrt(D))
                            if pair[0] >= 4 * qb:
                                # both blocks diagonal: one merged causal
                                # mask over the full tile (2D affine)
                                pt2v = pt2.rearrange(
                                    "p (j c) -> p j c", j=2)
                                nc.gpsimd.affine_select(
                                    out=pt2v, in_=pt2v,
                                    compare_op=Alu.is_ge, fill=0.0,
                                    base=qb * 512 - pair[0] * 128,
                                    channel_multiplier=-1,
                                    pattern=[[-128, 2], [1, 512]])
                            else:
                                for j, kb in enumerate(pair):
                                    if kb >= 4 * qb:  # diag: causal mask
                                        nc.gpsimd.affine_select(
                                            out=pt2[:,
                                                    j * 512:(j + 1) * 512],
                                            in_=pt2[:,
                                                    j * 512:(j + 1) * 512],
                                            compare_op=Alu.is_ge, fill=0.0,
                                            base=qb * 512 - kb * 128,
                                            channel_multiplier=-1,
                                            pattern=[[1, 512]])
                            inflight.append((kp, pair, pt2))
                            if kp == 0 and pend_den is not None:
                                emit_den(*pend_den)
                                pend_den = None
                            elif kp in y_slots and pend_y:
                                # spread the remaining pieces over the
                                # remaining slots of this qb
                                rem_slots = len(y_slots) - y_slots.index(kp) \
                                    + (len(y_slots) if h == 1 else 0)
                                take = -(-len(pend_y) // max(rem_slots, 1))
                                for _ in range(take):
                                    if pend_y:
                                        emit_y(*pend_y.pop(0))
                            if len(inflight) > 2:
                                emit_av(*inflight.pop(0))
                        for item in inflight:
                            emit_av(*item)
                        if pend_den is not None:   # corner safety
                            emit_den(*pend_den)
                        pend_den = (qb, h, ov)
                    pend_y = [(qb, mi, c0) for mi in range(4)
                              for c0 in (0, 384)]
                # drain the tail: last head's den chain + last qb's y
                emit_den(*pend_den)
                for piece in pend_y:
                    emit_y(*piece)

    _split_waits(nc)
    return nc


# ---------------------------------------------------------------- runner
class SpmdRunner:
    """Runs a Bass module on the first `n_cores` jax devices via the axon
    PJRT path (mirrors concourse.bass2jax.run_bass_via_pjrt, minus donation
    so the jitted callable is re-invocable for timing)."""

    def __init__(self, nc, n_cores=N_CORES):
        import jax
        import numpy as _np
        from jax.sharding import Mesh, PartitionSpec
        from jax.experimental.shard_map import shard_map
        import concourse.mybir as mybir
        from concourse.bass2jax import _bass_exec_p, install_neuronx_cc_hook

        install_neuronx_cc_hook()
        self.jax = jax
        self.n_cores = n_cores
        in_names, out_names, out_avals, zero_outs = [], [], [], []
        for alloc in nc.m.functions[0].allocations:
            if not isinstance(alloc, mybir.MemoryLocationSet):
                continue
            name = alloc.memorylocations[0].name
            if alloc.kind == "ExternalInput":
                in_names.append(name)
            elif alloc.kind == "ExternalOutput":
                shape = tuple(alloc.tensor_shape)
                dtype = mybir.dt.np(alloc.dtype)
                out_names.append(name)
                out_avals.append(jax.core.ShapedArray(shape, dtype))
                zero_outs.append(_np.zeros(shape, dtype))
        self.in_names, self.out_names = in_names, out_names
        self.out_avals, self.zero_outs = out_avals, zero_outs
        all_names = in_names + out_names

        def _body(*args):
            return tuple(_bass_exec_p.bind(
                *args,
                out_avals=tuple(out_avals),
                in_names=tuple(all_names),
                out_names=tuple(out_names),
                lowering_input_output_aliases=(),
                sim_require_finite=False,
                sim_require_nnan=False,
                nc=nc,
            ))

        devices = jax.devices()[:n_cores]
        self.mesh = Mesh(np.asarray(devices), ("core",))
        nin = len(in_names) + len(out_names)
        self.fn = jax.jit(
            shard_map(_body, mesh=self.mesh,
                      in_specs=(PartitionSpec("core"),) * nin,
                      out_specs=(PartitionSpec("core"),) * len(out_names),
                      check_rep=False),
            keep_unused=True,
        )
        self._dev_args = None

    def prepare(self, in_maps):
        import jax
        from jax.sharding import NamedSharding, PartitionSpec
        sharding = NamedSharding(self.mesh, PartitionSpec("core"))
        concat = [
            np.concatenate([np.ascontiguousarray(m[name]) for m in in_maps],
                           axis=0)
            for name in self.in_names
        ]
        concat += [
            np.zeros((self.n_cores * z.shape[0], *z.shape[1:]), z.dtype)
            for z in self.zero_outs
        ]
        self._dev_args = [jax.device_put(a, sharding) for a in concat]

    def run(self):
        outs = self.fn(*self._dev_args)
        self.jax.block_until_ready(outs)
        return [
            {name: np.asarray(outs[i]).reshape(
                self.n_cores, *self.out_avals[i].shape)[c]
             for i, name in enumerate(self.out_names)}
            for c in range(self.n_cores)
        ]


# ---------------------------------------------------------------- host side
def _rope_tables():
    inv_freq = 1.0 / (ROPE_BASE ** (np.arange(0, D, 2, dtype=np.float64) / D))
    t = np.arange(S, dtype=np.float64)
    freqs = np.outer(t, inv_freq)                      # [S, 32]
    emb = np.concatenate([freqs, freqs], axis=-1)      # [S, 64]
    cosT = np.cos(emb).T.astype(np.float32)            # [64, S]
    sinT = np.sin(emb).T.astype(np.float32)
    return (np.vstack([cosT, cosT]), np.vstack([sinT, sinT]))  # [128, S]


def _perm_mat():
    P = np.zeros((D, D), np.float32)
    for i in range(32):
        P[i, i + 32] = -1.0
        P[i + 32, i] = 1.0
    return P


def make_in_maps(x, Wq, bq, Wk, bk, Wv, bv, Wp, bp):
    import ml_dtypes
    bf16 = ml_dtypes.bfloat16
    cos2, sin2 = _rope_tables()
    trig = np.concatenate([cos2, sin2], axis=1).astype(bf16)   # [128, 4096]
    P = _perm_mat()
    P2 = np.zeros((128, 128), np.float32)
    P2[:64, :64] = P
    P2[64:, 64:] = P
    in_maps = []
    for c in range(N_CORES):
        b, g = c // 4, c % 4
        hs = slice(192 * g, 192 * (g + 1))
        wq_s, wk_s = Wq[:, hs], Wk[:, hs]
        # chunk0 = q heads 0,1 ; chunk1 = k heads 0,1 ; chunk2 = q2|k2
        wqk_s = np.concatenate(
            [wq_s[:, 0:128], wk_s[:, 0:128],
             wq_s[:, 128:192], wk_s[:, 128:192]], axis=1)       # [768, 384]
        bqk = np.concatenate(
            [bq[hs][0:128], bk[hs][0:128],
             bq[hs][128:192], bk[hs][128:192]])                 # [384]
        biases = np.zeros((128, 3), np.float32)
        for ch in range(3):
            biases[:, ch] = bqk[128 * ch:128 * (ch + 1)]
        wv_s = np.zeros((E, 256), np.float32)
        wv7 = np.zeros(256, np.float32)
        for h in range(HPC):
            wv_s[:, 65 * h:65 * h + 64] = \
                Wv[:, 192 * g + 64 * h:192 * g + 64 * (h + 1)]
            wv7[65 * h + 64] = 1.0
        smalls = np.zeros((128, 2432), np.float32)
        smalls[:, 0:128] = P2.T
        smalls[0, 128:128 + S] = 1.0
        smalls[0, 128 + S:128 + S + 256] = wv7
        in_maps.append({
            "xT": np.ascontiguousarray(x[b].T).astype(bf16),
            "wqk": wqk_s.astype(bf16),
            "wv": wv_s.astype(bf16),
            "wp": np.ascontiguousarray(Wp[hs, :]).astype(bf16),
            "biases": biases,
            "trig": trig,
            "smalls": smalls.astype(bf16),
        })
    return in_maps


def get_runner():
    global _RUNNER
    if _RUNNER is None:
        nc = build_bass()
        _RUNNER = SpmdRunner(nc, N_CORES)
    return _RUNNER


def assemble(results, Wp, bp, bv):
    y = np.zeros((B, S, E), np.float32)
    for c in range(N_CORES):
        y[c // 4] += results[c]["y"]
    y += (bp + bv @ Wp).astype(np.float32)
    return y


def kernel(x, Wq, bq, Wk, bk, Wv, bv, Wp, bp):
    runner = get_runner()
    runner.prepare(make_in_maps(x, Wq, bq, Wk, bk, Wv, bv, Wp, bp))
    return assemble(runner.run(), Wp, bp, bv)


# revision 61
# speedup vs baseline: 1.0908x; 1.0908x over previous
"""Causal multi-head attention with RoPE on 8 Trainium2 NeuronCores (Bass/Tile).

Problem: B=2, S=2048, E=768, H=12 heads, D=64, full rotary (ROPE_DIM=D),
causal softmax, fused QKV + output projection.

Sharding: 8 cores = 2 batches x 4 head-groups (3 heads each).

v2 (bf16, ~146-153us vs the 207us fp32r baseline):
  - all matmul inputs bf16 (PSUM accumulation stays fp32); y partials
    written back as bf16 and upcast on the host,
  - q&k projections fused into 3 full 128-partition chunks (2 head-dim
    blocks each: [q0|q1], [k0|k1], [q2|k2]) so neither the PE array nor
    the RoPE vector ops ever run at M=64; q2 re-copied to partition
    rows 64:128 (4x-mode DVE copy) so head 2's score matmul sees q and
    k at the same base partition,
  - q/k bias folded into the ScalarE PSUM->SBUF copy (per-partition
    bias operand) so RoPE is two 2x-mode bf16 DVE multiplies + one add,
  - attention per (query-block, head) unit with diagonal key-blocks
    first and the AV matmuls software-pipelined 2 score-tiles behind,
    so PE's in-order stream never blocks on the exp -> causal-mask
    chain (ScalarE exp -> gpsimd affine_select -> AV),
  - softmax denominator 1/den = exp(-ln den) on ScalarE (same
    activation table as the score exps -> no table reloads), partition-
    broadcast via a DRAM-roundtrip stride-0 DMA, multiply on DVE; the
    whole chain is emitted inside the NEXT unit's kp loop and each
    query block's y projection (bf16 half-row pieces) inside the NEXT
    block's heads 1-2 at kp>=2, so in-order engines never head-of-line
    block on them,
  - x streamed in eighth-column chunks and wqk split per chunk so the
    first projection matmul starts ~4us in.
Rejected on HW measurement: K=1 ones-matmul broadcast of 1/den
(PE slow path), DVE InstReciprocal (~5x slower than modeled), moving
the half-1 projections into the attention phase (pt/s2 pool
contention), gpsimd partition_broadcast + custom-DVE ops (this
walrus can't codegen InstISA), MAX_WAITS>1 (walrus cap).
Host sums the 4 partials per batch and adds (bp + bv @ Wp) once.
"""
import math

import numpy as np

N_HEADS = 12
ROPE_BASE = 10000.0
B, S, E = 2, 2048, 768
D = 64
HPC = 3            # heads per core
N_CORES = 8
QB = 512           # query block (free dim of score tiles)
KB = 128           # key block (partition dim of score tiles)
NQB = S // QB      # 4
NKB = S // KB      # 16
EK = E // 128      # 6 contraction chunks

_RUNNER = None


# ---------------------------------------------------------------- tile patch
def _patch_tile_drain():
    """This container's walrus caps semaphore waits per instruction ("Too
    many sync wait commands").  Split the TileContext tail-drain waits
    across dedicated SP nops."""
    import concourse.tile as tile
    import concourse.mybir as mybir

    if getattr(tile.TileContext, "_drain_patched", False):
        return

    def _drain_and_barrier(self, tick_clock, wait_clock):
        nc = self.nc
        drain_inst = nc.sync.drain()
        wait_clock.add_sem_waits(
            drain_inst.ins, tile.ScopedClock({None: tick_clock.global_clock})
        )
        si = drain_inst.ins.sync_info
        waits = list(si.on_wait) if si is not None else []
        if len(waits) > 1:
            drain_inst.ins.sync_info.on_wait = waits[:1]
            for w in waits[1:]:
                n = nc.sync.nop(nofuse=True)
                n.ins.sync_info = mybir.SyncInfo(on_wait=[w], on_update=[])
        nc.all_engine_barrier()
        assert self.sems is not None
        popped = nc._tile_sem_poison_stack.pop()
        assert popped is self._sem_poison
        nc.clear_and_free_semaphores(list(self.sems.allocated().values()))
        nc.all_engine_barrier()

    tile.TileContext._drain_and_barrier = _drain_and_barrier
    tile.TileContext._drain_patched = True


MAX_WAITS = 1


def _split_waits(nc, maxw=None):
    """Move excess semaphore waits onto same-engine NoOps inserted just
    before the carrying instruction (walrus per-instruction wait cap)."""
    import concourse.mybir as mybir

    if maxw is None:
        maxw = MAX_WAITS
    k = 0
    for f in nc.m.functions:
        for bb in f.blocks:
            new = []
            for ins in bb.instructions:
                si = ins.sync_info
                if si is not None and len(si.on_wait) > maxw:
                    waits = list(si.on_wait)
                    head, tail = waits[:-maxw], waits[-maxw:]
                    for i in range(0, len(head), maxw):
                        nop = mybir.InstNoOp(
                            name=f"{ins.name}-sw{k}", ins=[], outs=[])
                        k += 1
                        nop.engine = ins.engine
                        nop.sync_info = mybir.SyncInfo(
                            on_wait=head[i:i + maxw], on_update=[])
                        new.append(nop)
                    si.on_wait = tail
                new.append(ins)
            bb.instructions = new


# ---------------------------------------------------------------- device IR
def build_bass(reps=1):
    """reps>1 wraps the whole kernel in an on-device For_i repeat loop --
    used only for timing (slope between rep counts removes dispatch
    overhead)."""
    import contextlib
    import concourse.bass as bass
    import concourse.mybir as mybir
    import concourse.tile as tile

    _patch_tile_drain()
    f32 = mybir.dt.float32
    bf16 = mybir.dt.bfloat16
    Act = mybir.ActivationFunctionType
    Alu = mybir.AluOpType

    nc = bass.Bass(enable_partition_id=False)
    xT = nc.dram_tensor("xT", [E, S], bf16, kind="ExternalInput")
    wqk = nc.dram_tensor("wqk", [E, 384], bf16, kind="ExternalInput")
    wv = nc.dram_tensor("wv", [E, 256], bf16, kind="ExternalInput")
    wp = nc.dram_tensor("wp", [HPC * D, E], bf16, kind="ExternalInput")
    biasesd = nc.dram_tensor("biases", [128, 3], f32, kind="ExternalInput")
    trigd = nc.dram_tensor("trig", [128, 2 * S], bf16, kind="ExternalInput")
    smallsd = nc.dram_tensor("smalls", [128, 2432], bf16, kind="ExternalInput")
    dend = nc.dram_tensor("dend", [12, 512], f32, kind="Internal")
    y = nc.dram_tensor("y", [S, E], bf16, kind="ExternalOutput")

    with tile.TileContext(nc) as tc:
        rep_loop = (
            tc.For_i(0, reps, 1,
                     hint_engines=(mybir.EngineType.PE, mybir.EngineType.DVE,
                                   mybir.EngineType.Activation,
                                   mybir.EngineType.Pool, mybir.EngineType.SP))
            if reps > 1 else contextlib.nullcontext()
        )
        with rep_loop, (
            tc.tile_pool(name="consts", bufs=1)
        ) as consts, tc.tile_pool(name="big", bufs=1) as big:
            # ---- constant loads.  wqk first (gates the first matmul),
            # then x in quarter-column chunks so compute starts early.
            wqk_all = consts.tile([128, EK * 384], bf16, tag="wqk_all")
            wqk_s = wqk_all.rearrange("p (a m) -> p a m", a=EK)
            wqk_d = wqk.rearrange("(a p) m -> p a m", p=128)
            nc.sync.dma_start(out=wqk_s[:, :, 0:128], in_=wqk_d[:, :, 0:128])
            wqk_t = [wqk_all[:, e * 384:(e + 1) * 384] for e in range(EK)]
            biases_t = consts.tile([128, 3], f32, tag="biases")
            nc.sync.dma_start(out=biases_t, in_=biasesd[:, :])
            xt_all = big.tile([128, EK * S], bf16, tag="xt_all")
            xt3 = xt_all.rearrange("p (a m) -> p a m", a=EK)
            xs3 = xT.rearrange("(a p) m -> p a m", p=128)

            def load_x(i8, n=1):
                nc.sync.dma_start(
                    out=xt3[:, :, i8 * 256:(i8 + n) * 256],
                    in_=xs3[:, :, i8 * 256:(i8 + n) * 256])

            load_x(0)
            load_x(1)
            nc.sync.dma_start(out=wqk_s[:, :, 128:384],
                              in_=wqk_d[:, :, 128:384])
            xt = [xt_all[:, e * S:(e + 1) * S] for e in range(EK)]
            smalls_t = consts.tile([128, 2432], bf16, tag="smalls")
            nc.sync.dma_start(out=smalls_t, in_=smallsd[:, :])
            p2_t = smalls_t[:, 0:128]
            ones_row = smalls_t[0:1, 128:128 + S]
            wv7 = smalls_t[0:1, 128 + S:128 + S + 256]
            wv_all = consts.tile([128, EK * 256], bf16, tag="wv_all")
            nc.sync.dma_start(
                out=wv_all.rearrange("p (a m) -> p a m", a=EK),
                in_=wv.rearrange("(a p) m -> p a m", p=128))
            wv_t = [wv_all[:, e * 256:(e + 1) * 256] for e in range(EK)]
            load_x(2, 2)
            trig_t = consts.tile([128, 2 * S], bf16, tag="trig")
            nc.sync.dma_start(out=trig_t, in_=trigd[:, :])
            cos_t = trig_t[:, 0:S]
            sin_t = trig_t[:, S:2 * S]
            load_x(4, 2)
            load_x(6, 2)
            wp0 = consts.tile([128, E], bf16, tag="wp0")
            nc.sync.dma_start(out=wp0, in_=wp[0:128, :])
            wp1 = consts.tile([64, E], bf16, tag="wp1")
            nc.sync.dma_start(out=wp1, in_=wp[128:192, :])

            # ---- long-lived activations: 3 fused q|k chunks, each 128
            # partitions = 2 head-dim blocks of 64.
            # chunk0 = q heads 0,1 ; chunk1 = k heads 0,1 ;
            # chunk2 = q head 2 | k head 2.  Scores need lhsT/rhs at the
            # same base partition, so q2 is re-copied to rows 64:128 of a
            # scratch tile (4x-mode DVE copy).
            qk_c = [big.tile([128, S], bf16, tag=f"qk{c}", name=f"qk{c}")
                    for c in range(3)]
            q2scr = big.tile([128, S], bf16, tag="q2scr", name="q2scr")
            v2_sb = [big.tile([128, 512], bf16, tag=f"v2_{s}", name=f"v2_{s}")
                     for s in range(NKB // 2)]
            oTa_q = [big.tile([128, QB], bf16, tag=f"oTa{qb}",
                              name=f"oTa{qb}") for qb in range(NQB)]
            oTb_q = [big.tile([64, QB], bf16, tag=f"oTb{qb}",
                              name=f"oTb{qb}") for qb in range(NQB)]

            # ============================ phase 1: projections + RoPE
            with (
                tc.tile_pool(name="psq", bufs=2, space="PSUM") as psq_pool,
                tc.tile_pool(name="psrot", bufs=1, space="PSUM") as rot_pool,
                tc.tile_pool(name="psv", bufs=2, space="PSUM") as psv_pool,
                tc.tile_pool(name="ropetmp", bufs=2) as rtmp,
            ):
                def emit_qk_chunk(ch, half, pools=None):
                    ps_pool, r_pool, t_pool, ps_tag, t_tag = pools or (
                        psq_pool, rot_pool, rtmp, "psq", None)
                    c0 = half * 1024
                    ps = ps_pool.tile([128, 1024], f32, tag=ps_tag)
                    for n in range(4):
                        for e in range(EK):
                            nc.tensor.matmul(
                                ps[:, n * 256:(n + 1) * 256],
                                lhsT=wqk_t[e][:, ch * 128:(ch + 1) * 128],
                                rhs=xt[e][:, c0 + n * 256:c0 + (n + 1) * 256],
                                start=(e == 0), stop=(e == EK - 1),
                            )
                    # biased q -> SBUF bf16 via ScalarE (bias per partition)
                    q_sb = t_pool.tile([128, 1024], bf16,
                                       tag=t_tag or "qsb")
                    nc.scalar.activation(q_sb, ps, Act.Identity,
                                         bias=biases_t[:, ch:ch + 1])
                    # qc = q_sb * cos   (all-bf16: 2x DVE mode)
                    qc = t_pool.tile([128, 1024], bf16, tag=t_tag or "qc")
                    nc.vector.tensor_mul(qc, q_sb, cos_t[:, c0:c0 + 1024])
                    # qrot = P2 @ q_sb ; qs = qrot * sin ; out = qc + qs
                    rot = r_pool.tile([128, 1024], f32,
                                      tag="rot" if pools is None else ps_tag)
                    for n in range(2):
                        nc.tensor.matmul(
                            rot[:, n * 512:(n + 1) * 512], lhsT=p2_t,
                            rhs=q_sb[:, n * 512:(n + 1) * 512],
                            start=True, stop=True)
                    qs = t_pool.tile([128, 1024], bf16, tag=t_tag or "qs")
                    nc.vector.tensor_mul(qs, rot, sin_t[:, c0:c0 + 1024])
                    nc.vector.tensor_add(qk_c[ch][:, c0:c0 + 1024], qc, qs)
                    if ch == 2:
                        nc.vector.tensor_copy(
                            q2scr[64:128, c0:c0 + 1024],
                            qk_c[2][0:64, c0:c0 + 1024])

                def emit_v_block(s0):
                    # two sblocks share one PSUM bank; one copy out
                    ps = psv_pool.tile([128, 512], f32, tag="psv")
                    for i in range(2):
                        s = s0 + i
                        c = i * 256
                        for e in range(EK):
                            nc.tensor.matmul(
                                ps[:, c:c + 256],
                                lhsT=xt[e][:, s * 128:(s + 1) * 128],
                                rhs=wv_t[e], start=(e == 0), stop=False)
                        nc.tensor.matmul(
                            ps[:, c:c + 256],
                            lhsT=ones_row[:, s * 128:(s + 1) * 128],
                            rhs=wv7, start=False, stop=True)
                    nc.vector.tensor_copy(v2_sb[s0 // 2], ps)

                # half 0 of all 3 chunks first so attention and the
                # second x half DMA overlap phase 1's tail.
                chunks = [(ch, half) for half in range(2) for ch in range(3)]
                vs = iter(range(0, NKB, 2))
                for ch, half in chunks:
                    emit_qk_chunk(ch, half)
                    s = next(vs, None)
                    if s is not None:
                        emit_v_block(s)
                for s in vs:
                    emit_v_block(s)

            # ============================ phase 2+3: attention + y proj
            def v_lhsT(s, h):
                # head values cols [65h..65h+63] + ones col at 65h+64
                return v2_sb[s // 2][:, (s % 2) * 256 + 65 * h:
                                     (s % 2) * 256 + 65 * h + 65]

            # per-head (qT tile, kT tile, partition row for both)
            heads = ((qk_c[0], qk_c[1], 0),
                     (qk_c[0], qk_c[1], 64),
                     (q2scr, qk_c[2], 64))

            with (
                tc.tile_pool(name="ps_s", bufs=2, space="PSUM") as s_pool,
                tc.tile_pool(name="ps_ov", bufs=2, space="PSUM") as ov_pool,
                tc.tile_pool(name="ps_y", bufs=2, space="PSUM") as y_pool,
                tc.tile_pool(name="pt", bufs=6) as pt_pool,
                tc.tile_pool(name="eps", bufs=2) as ep_pool,
                tc.tile_pool(name="ysb", bufs=2) as ysb_pool,
            ):
                def emit_den(qb, h, ov):
                    # normalize, inside the NEXT unit: 1/den = exp(-ln den)
                    # on ScalarE (same activation table as the score exps),
                    # partition-broadcast via a DRAM roundtrip (stride-0
                    # read), multiply on DVE
                    import concourse.bass as _b
                    dl = ep_pool.tile([1, 512], f32, tag="dl")
                    nc.scalar.activation(dl, ov[64:65, :], Act.Ln)
                    rec_sb = ep_pool.tile([1, 512], f32, tag="den")
                    nc.scalar.activation(rec_sb, dl, Act.Exp, scale=-1.0)
                    ei = 3 * qb + h
                    nc.sync.dma_start(out=dend[ei:ei + 1, :], in_=rec_sb)
                    recb = ep_pool.tile([64, 512], f32, tag="denb")
                    dsrc = dend[ei:ei + 1, :]
                    nc.sync.dma_start(
                        out=recb,
                        in_=_b.AP(tensor=dsrc.tensor, offset=dsrc.offset,
                                  ap=[[0, 64]] + list(dsrc.ap[1:])))
                    if h < 2:
                        dst = oTa_q[qb][64 * h:64 * h + 64, :]
                    else:
                        dst = oTb_q[qb]
                    nc.vector.tensor_mul(dst, ov[0:64, :], recb)

                def emit_y(qb, mi, c0):
                    # one half-row-block piece: [128 queries, 384 cols]
                    m = 4 * qb + mi
                    yp = y_pool.tile([128, 384], f32, tag="yp")
                    nc.tensor.matmul(
                        yp,
                        lhsT=oTa_q[qb][:, mi * 128:(mi + 1) * 128],
                        rhs=wp0[:, c0:c0 + 384],
                        start=True, stop=False)
                    nc.tensor.matmul(
                        yp,
                        lhsT=oTb_q[qb][:, mi * 128:(mi + 1) * 128],
                        rhs=wp1[:, c0:c0 + 384],
                        start=False, stop=True)
                    y_sb = ysb_pool.tile([128, 384], bf16, tag="ysb")
                    nc.vector.tensor_copy(y_sb, yp)
                    nc.sync.dma_start(
                        out=y[m * 128:(m + 1) * 128, c0:c0 + 384], in_=y_sb)

                # Software pipeline: each head's den-chain (DVE/Pool only)
                # is emitted inside the NEXT head's kp loop, and each query
                # block's y projection inside the NEXT block's heads, so
                # in-order engines never head-of-line block on them.
                pend_den = None           # (qb, h, ov) awaiting den chain
                pend_y = []               # [(qb, mi)] awaiting y projection
                for qb in range(NQB):
                    for h, (qt, kt, pr) in enumerate(heads):
                        ov = ov_pool.tile([128, 512], f32, tag="ov")
                        qslice = qt[pr:pr + 64, qb * 512:(qb + 1) * 512]
                        nkb = 4 * (qb + 1)
                        # diagonal blocks first: their mask latency hides
                        # behind the past-key matmuls that follow
                        kbs = list(range(4 * qb, nkb)) + list(range(4 * qb))
                        # y half-pieces of the previous qb go into heads
                        # 1-2 at kp>=2, far enough from the den DMA
                        # roundtrip they depend on
                        y_slots = list(range(2, nkb // 2)) if h >= 1 else []
                        def emit_av(kp, pair, pt2):
                            for j, kb in enumerate(pair):
                                nc.tensor.matmul(
                                    ov[0:65, :], lhsT=v_lhsT(kb, h),
                                    rhs=pt2[:, j * 512:(j + 1) * 512],
                                    start=(2 * kp + j == 0),
                                    stop=(2 * kp + j == nkb - 1))

                        # AV runs 2 kp-steps behind scores so PE's in-order
                        # stream never blocks on the exp -> mask chain
                        inflight = []
                        for kp in range(nkb // 2):
                            pair = kbs[2 * kp:2 * kp + 2]
                            s2 = s_pool.tile([128, 1024], f32, tag="s2")
                            pt2 = pt_pool.tile([128, 1024], bf16, tag="pt2")
                            for j, kb in enumerate(pair):
                                nc.tensor.matmul(
                                    s2[:, j * 512:(j + 1) * 512],
                                    lhsT=kt[pr:pr + 64,
                                            kb * 128:(kb + 1) * 128],
                                    rhs=qslice, start=True, stop=True)
                            nc.scalar.activation(
                                pt2, s2, Act.Exp, scale=1.0 / math.sqrt(D))
                            if pair[0] >= 4 * qb:
                                # both blocks diagonal: one merged causal
                                # mask over the full tile (2D affine)
                                pt2v = pt2.rearrange(
                                    "p (j c) -> p j c", j=2)
                                nc.gpsimd.affine_select(
                                    out=pt2v, in_=pt2v,
                                    compare_op=Alu.is_ge, fill=0.0,
                                    base=qb * 512 - pair[0] * 128,
                                    channel_multiplier=-1,
                                    pattern=[[-128, 2], [1, 512]])
                            else:
                                for j, kb in enumerate(pair):
                                    if kb >= 4 * qb:  # diag: causal mask
                                        nc.gpsimd.affine_select(
                                            out=pt2[:,
                                                    j * 512:(j + 1) * 512],
                                            in_=pt2[:,
                                                    j * 512:(j + 1) * 512],
                                            compare_op=Alu.is_ge, fill=0.0,
                                            base=qb * 512 - kb * 128,
                                            channel_multiplier=-1,
                                            pattern=[[1, 512]])
                            inflight.append((kp, pair, pt2))
                            if kp == 0 and pend_den is not None:
                                emit_den(*pend_den)
                                pend_den = None
                            elif kp in y_slots and pend_y:
                                # spread the remaining pieces over the
                                # remaining slots of this qb
                                rem_slots = len(y_slots) - y_slots.index(kp) \
                                    + (len(y_slots) if h == 1 else 0)
                                take = -(-len(pend_y) // max(rem_slots, 1))
                                for _ in range(take):
                                    if pend_y:
                                        emit_y(*pend_y.pop(0))
                            if len(inflight) > 2:
                                emit_av(*inflight.pop(0))
                        for item in inflight:
                            emit_av(*item)
                        if pend_den is not None:   # corner safety
                            emit_den(*pend_den)
                        pend_den = (qb, h, ov)
                    pend_y = [(qb, mi, c0) for mi in range(4)
                              for c0 in (0, 384)]
                # drain the tail: last head's den chain + last qb's y
                emit_den(*pend_den)
                for piece in pend_y:
                    emit_y(*piece)

    _split_waits(nc)
    return nc


# ---------------------------------------------------------------- runner
class SpmdRunner:
    """Runs a Bass module on the first `n_cores` jax devices via the axon
    PJRT path (mirrors concourse.bass2jax.run_bass_via_pjrt, minus donation
    so the jitted callable is re-invocable for timing)."""

    def __init__(self, nc, n_cores=N_CORES):
        import jax
        import numpy as _np
        from jax.sharding import Mesh, PartitionSpec
        from jax.experimental.shard_map import shard_map
        import concourse.mybir as mybir
        from concourse.bass2jax import _bass_exec_p, install_neuronx_cc_hook

        install_neuronx_cc_hook()
        self.jax = jax
        self.n_cores = n_cores
        in_names, out_names, out_avals, zero_outs = [], [], [], []
        for alloc in nc.m.functions[0].allocations:
            if not isinstance(alloc, mybir.MemoryLocationSet):
                continue
            name = alloc.memorylocations[0].name
            if alloc.kind == "ExternalInput":
                in_names.append(name)
            elif alloc.kind == "ExternalOutput":
                shape = tuple(alloc.tensor_shape)
                dtype = mybir.dt.np(alloc.dtype)
                out_names.append(name)
                out_avals.append(jax.core.ShapedArray(shape, dtype))
                zero_outs.append(_np.zeros(shape, dtype))
        self.in_names, self.out_names = in_names, out_names
        self.out_avals, self.zero_outs = out_avals, zero_outs
        all_names = in_names + out_names

        def _body(*args):
            return tuple(_bass_exec_p.bind(
                *args,
                out_avals=tuple(out_avals),
                in_names=tuple(all_names),
                out_names=tuple(out_names),
                lowering_input_output_aliases=(),
                sim_require_finite=False,
                sim_require_nnan=False,
                nc=nc,
            ))

        devices = jax.devices()[:n_cores]
        self.mesh = Mesh(np.asarray(devices), ("core",))
        nin = len(in_names) + len(out_names)
        self.fn = jax.jit(
            shard_map(_body, mesh=self.mesh,
                      in_specs=(PartitionSpec("core"),) * nin,
                      out_specs=(PartitionSpec("core"),) * len(out_names),
                      check_rep=False),
            keep_unused=True,
        )
        self._dev_args = None

    def prepare(self, in_maps):
        import jax
        from jax.sharding import NamedSharding, PartitionSpec
        sharding = NamedSharding(self.mesh, PartitionSpec("core"))
        concat = [
            np.concatenate([np.ascontiguousarray(m[name]) for m in in_maps],
                           axis=0)
            for name in self.in_names
        ]
        concat += [
            np.zeros((self.n_cores * z.shape[0], *z.shape[1:]), z.dtype)
            for z in self.zero_outs
        ]
        self._dev_args = [jax.device_put(a, sharding) for a in concat]

    def run(self):
        outs = self.fn(*self._dev_args)
        self.jax.block_until_ready(outs)
        return [
            {name: np.asarray(outs[i]).reshape(
                self.n_cores, *self.out_avals[i].shape)[c]
             for i, name in enumerate(self.out_names)}
            for c in range(self.n_cores)
        ]


# ---------------------------------------------------------------- host side
def _rope_tables():
    inv_freq = 1.0 / (ROPE_BASE ** (np.arange(0, D, 2, dtype=np.float64) / D))
    t = np.arange(S, dtype=np.float64)
    freqs = np.outer(t, inv_freq)                      # [S, 32]
    emb = np.concatenate([freqs, freqs], axis=-1)      # [S, 64]
    cosT = np.cos(emb).T.astype(np.float32)            # [64, S]
    sinT = np.sin(emb).T.astype(np.float32)
    return (np.vstack([cosT, cosT]), np.vstack([sinT, sinT]))  # [128, S]


def _perm_mat():
    P = np.zeros((D, D), np.float32)
    for i in range(32):
        P[i, i + 32] = -1.0
        P[i + 32, i] = 1.0
    return P


def make_in_maps(x, Wq, bq, Wk, bk, Wv, bv, Wp, bp):
    import ml_dtypes
    bf16 = ml_dtypes.bfloat16
    cos2, sin2 = _rope_tables()
    trig = np.concatenate([cos2, sin2], axis=1).astype(bf16)   # [128, 4096]
    P = _perm_mat()
    P2 = np.zeros((128, 128), np.float32)
    P2[:64, :64] = P
    P2[64:, 64:] = P
    in_maps = []
    for c in range(N_CORES):
        b, g = c // 4, c % 4
        hs = slice(192 * g, 192 * (g + 1))
        wq_s, wk_s = Wq[:, hs], Wk[:, hs]
        # chunk0 = q heads 0,1 ; chunk1 = k heads 0,1 ; chunk2 = q2|k2
        wqk_s = np.concatenate(
            [wq_s[:, 0:128], wk_s[:, 0:128],
             wq_s[:, 128:192], wk_s[:, 128:192]], axis=1)       # [768, 384]
        bqk = np.concatenate(
            [bq[hs][0:128], bk[hs][0:128],
             bq[hs][128:192], bk[hs][128:192]])                 # [384]
        biases = np.zeros((128, 3), np.float32)
        for ch in range(3):
            biases[:, ch] = bqk[128 * ch:128 * (ch + 1)]
        wv_s = np.zeros((E, 256), np.float32)
        wv7 = np.zeros(256, np.float32)
        for h in range(HPC):
            wv_s[:, 65 * h:65 * h + 64] = \
                Wv[:, 192 * g + 64 * h:192 * g + 64 * (h + 1)]
            wv7[65 * h + 64] = 1.0
        smalls = np.zeros((128, 2432), np.float32)
        smalls[:, 0:128] = P2.T
        smalls[0, 128:128 + S] = 1.0
        smalls[0, 128 + S:128 + S + 256] = wv7
        in_maps.append({
            "xT": np.ascontiguousarray(x[b].T).astype(bf16),
            "wqk": wqk_s.astype(bf16),
            "wv": wv_s.astype(bf16),
            "wp": np.ascontiguousarray(Wp[hs, :]).astype(bf16),
            "biases": biases,
            "trig": trig,
            "smalls": smalls.astype(bf16),
        })
    return in_maps


def get_runner():
    global _RUNNER
    if _RUNNER is None:
        nc = build_bass()
        _RUNNER = SpmdRunner(nc, N_CORES)
    return _RUNNER


def assemble(results, Wp, bp, bv):
    y = np.zeros((B, S, E), np.float32)
    for c in range(N_CORES):
        y[c // 4] += results[c]["y"]
    y += (bp + bv @ Wp).astype(np.float32)
    return y


def kernel(x, Wq, bq, Wk, bk, Wv, bv, Wp, bp):
    runner = get_runner()
    runner.prepare(make_in_maps(x, Wq, bq, Wk, bk, Wv, bv, Wp, bp))
    return assemble(runner.run(), Wp, bp, bv)


# revision 63
# speedup vs baseline: 1.1721x; 1.0745x over previous
"""Causal multi-head attention with RoPE on 8 Trainium2 NeuronCores (Bass/Tile).

Problem: B=2, S=2048, E=768, H=12 heads, D=64, full rotary (ROPE_DIM=D),
causal softmax, fused QKV + output projection.

Sharding: 8 cores = 2 batches x 4 head-groups (3 heads each).

v2 (bf16, ~146-153us vs the 207us fp32r baseline):
  - all matmul inputs bf16 (PSUM accumulation stays fp32); y partials
    written back as bf16 and upcast on the host,
  - q&k projections fused into 3 full 128-partition chunks (2 head-dim
    blocks each: [q0|q1], [k0|k1], [q2|k2]) so neither the PE array nor
    the RoPE vector ops ever run at M=64; q2 re-copied to partition
    rows 64:128 (4x-mode DVE copy) so head 2's score matmul sees q and
    k at the same base partition,
  - q/k bias folded into the ScalarE PSUM->SBUF copy (per-partition
    bias operand) so RoPE is two 2x-mode bf16 DVE multiplies + one add,
  - attention per (query-block, head) unit with diagonal key-blocks
    first and the AV matmuls software-pipelined 2 score-tiles behind,
    so PE's in-order stream never blocks on the exp -> causal-mask
    chain (ScalarE exp -> gpsimd affine_select -> AV),
  - softmax denominator 1/den = exp(-ln den) on ScalarE (same
    activation table as the score exps -> no table reloads), partition-
    broadcast via a DRAM-roundtrip stride-0 DMA, multiply on DVE; the
    whole chain is emitted inside the NEXT unit's kp loop and each
    query block's y projection (bf16 half-row pieces) inside the NEXT
    block's heads 1-2 at kp>=2, so in-order engines never head-of-line
    block on them,
  - x streamed in eighth-column chunks and wqk split per chunk so the
    first projection matmul starts ~4us in.
Rejected on HW measurement: K=1 ones-matmul broadcast of 1/den
(PE slow path), DVE InstReciprocal (~5x slower than modeled), moving
the half-1 projections into the attention phase (pt/s2 pool
contention), gpsimd partition_broadcast + custom-DVE ops (this
walrus can't codegen InstISA), MAX_WAITS>1 (walrus cap).
Host sums the 4 partials per batch and adds (bp + bv @ Wp) once.
"""
import math

import numpy as np

N_HEADS = 12
ROPE_BASE = 10000.0
B, S, E = 2, 2048, 768
D = 64
HPC = 3            # heads per core
N_CORES = 8
QB = 512           # query block (free dim of score tiles)
KB = 128           # key block (partition dim of score tiles)
NQB = S // QB      # 4
NKB = S // KB      # 16
EK = E // 128      # 6 contraction chunks

_RUNNER = None


# ---------------------------------------------------------------- tile patch
def _patch_tile_drain():
    """This container's walrus caps semaphore waits per instruction ("Too
    many sync wait commands").  Split the TileContext tail-drain waits
    across dedicated SP nops."""
    import concourse.tile as tile
    import concourse.mybir as mybir

    if getattr(tile.TileContext, "_drain_patched", False):
        return

    def _drain_and_barrier(self, tick_clock, wait_clock):
        nc = self.nc
        drain_inst = nc.sync.drain()
        wait_clock.add_sem_waits(
            drain_inst.ins, tile.ScopedClock({None: tick_clock.global_clock})
        )
        si = drain_inst.ins.sync_info
        waits = list(si.on_wait) if si is not None else []
        if len(waits) > 1:
            drain_inst.ins.sync_info.on_wait = waits[:1]
            for w in waits[1:]:
                n = nc.sync.nop(nofuse=True)
                n.ins.sync_info = mybir.SyncInfo(on_wait=[w], on_update=[])
        nc.all_engine_barrier()
        assert self.sems is not None
        popped = nc._tile_sem_poison_stack.pop()
        assert popped is self._sem_poison
        nc.clear_and_free_semaphores(list(self.sems.allocated().values()))
        nc.all_engine_barrier()

    tile.TileContext._drain_and_barrier = _drain_and_barrier
    tile.TileContext._drain_patched = True


MAX_WAITS = 1


def _split_waits(nc, maxw=None):
    """Move excess semaphore waits onto same-engine NoOps inserted just
    before the carrying instruction (walrus per-instruction wait cap)."""
    import concourse.mybir as mybir

    if maxw is None:
        maxw = MAX_WAITS
    k = 0
    for f in nc.m.functions:
        for bb in f.blocks:
            new = []
            for ins in bb.instructions:
                si = ins.sync_info
                if si is not None and len(si.on_wait) > maxw:
                    waits = list(si.on_wait)
                    head, tail = waits[:-maxw], waits[-maxw:]
                    for i in range(0, len(head), maxw):
                        nop = mybir.InstNoOp(
                            name=f"{ins.name}-sw{k}", ins=[], outs=[])
                        k += 1
                        nop.engine = ins.engine
                        nop.sync_info = mybir.SyncInfo(
                            on_wait=head[i:i + maxw], on_update=[])
                        new.append(nop)
                    si.on_wait = tail
                new.append(ins)
            bb.instructions = new


# ---------------------------------------------------------------- device IR
def build_bass(reps=1):
    """reps>1 wraps the whole kernel in an on-device For_i repeat loop --
    used only for timing (slope between rep counts removes dispatch
    overhead)."""
    import contextlib
    import concourse.bass as bass
    import concourse.mybir as mybir
    import concourse.tile as tile

    _patch_tile_drain()
    f32 = mybir.dt.float32
    bf16 = mybir.dt.bfloat16
    Act = mybir.ActivationFunctionType
    Alu = mybir.AluOpType

    nc = bass.Bass(enable_partition_id=False)
    xT = nc.dram_tensor("xT", [E, S], bf16, kind="ExternalInput")
    wqk = nc.dram_tensor("wqk", [E, 384], bf16, kind="ExternalInput")
    wv = nc.dram_tensor("wv", [E, 256], bf16, kind="ExternalInput")
    wp = nc.dram_tensor("wp", [HPC * D, E], bf16, kind="ExternalInput")
    biasesd = nc.dram_tensor("biases", [128, 3], f32, kind="ExternalInput")
    trigd = nc.dram_tensor("trig", [128, 2 * S], bf16, kind="ExternalInput")
    smallsd = nc.dram_tensor("smalls", [128, 2432], bf16, kind="ExternalInput")
    dend = nc.dram_tensor("dend", [12, 512], f32, kind="Internal")
    y = nc.dram_tensor("y", [S, E], bf16, kind="ExternalOutput")

    with tile.TileContext(nc) as tc:
        rep_loop = (
            tc.For_i(0, reps, 1,
                     hint_engines=(mybir.EngineType.PE, mybir.EngineType.DVE,
                                   mybir.EngineType.Activation,
                                   mybir.EngineType.Pool, mybir.EngineType.SP))
            if reps > 1 else contextlib.nullcontext()
        )
        with rep_loop, (
            tc.tile_pool(name="consts", bufs=1)
        ) as consts, tc.tile_pool(name="big", bufs=1) as big:
            # ---- constant loads.  wqk first (gates the first matmul),
            # then x in quarter-column chunks so compute starts early.
            wqk_all = consts.tile([128, EK * 384], bf16, tag="wqk_all")
            wqk_s = wqk_all.rearrange("p (a m) -> p a m", a=EK)
            wqk_d = wqk.rearrange("(a p) m -> p a m", p=128)
            nc.sync.dma_start(out=wqk_s[:, :, 0:128], in_=wqk_d[:, :, 0:128])
            wqk_t = [wqk_all[:, e * 384:(e + 1) * 384] for e in range(EK)]
            biases_t = consts.tile([128, 3], f32, tag="biases")
            nc.sync.dma_start(out=biases_t, in_=biasesd[:, :])
            xt_all = big.tile([128, EK * S], bf16, tag="xt_all")
            xt3 = xt_all.rearrange("p (a m) -> p a m", a=EK)
            xs3 = xT.rearrange("(a p) m -> p a m", p=128)

            def load_x(i8, n=1):
                nc.sync.dma_start(
                    out=xt3[:, :, i8 * 256:(i8 + n) * 256],
                    in_=xs3[:, :, i8 * 256:(i8 + n) * 256])

            load_x(0)
            load_x(1)
            nc.sync.dma_start(out=wqk_s[:, :, 128:384],
                              in_=wqk_d[:, :, 128:384])
            xt = [xt_all[:, e * S:(e + 1) * S] for e in range(EK)]
            smalls_t = consts.tile([128, 2432], bf16, tag="smalls")
            nc.sync.dma_start(out=smalls_t, in_=smallsd[:, :])
            p2_t = smalls_t[:, 0:128]
            ones_row = smalls_t[0:1, 128:128 + S]
            wv7 = smalls_t[0:1, 128 + S:128 + S + 256]
            wv_all = consts.tile([128, EK * 256], bf16, tag="wv_all")
            nc.sync.dma_start(
                out=wv_all.rearrange("p (a m) -> p a m", a=EK),
                in_=wv.rearrange("(a p) m -> p a m", p=128))
            wv_t = [wv_all[:, e * 256:(e + 1) * 256] for e in range(EK)]
            load_x(2, 2)
            trig_t = consts.tile([128, 2 * S], bf16, tag="trig")
            nc.sync.dma_start(out=trig_t, in_=trigd[:, :])
            cos_t = trig_t[:, 0:S]
            sin_t = trig_t[:, S:2 * S]
            load_x(4, 2)
            load_x(6, 2)
            wp0 = consts.tile([128, E], bf16, tag="wp0")
            nc.sync.dma_start(out=wp0, in_=wp[0:128, :])
            wp1 = consts.tile([64, E], bf16, tag="wp1")
            nc.sync.dma_start(out=wp1, in_=wp[128:192, :])

            # ---- long-lived activations: 3 fused q|k chunks, each 128
            # partitions = 2 head-dim blocks of 64.
            # chunk0 = q heads 0,1 ; chunk1 = k heads 0,1 ;
            # chunk2 = q head 2 | k head 2.  Scores need lhsT/rhs at the
            # same base partition, so q2 is re-copied to rows 64:128 of a
            # scratch tile (4x-mode DVE copy).
            qk_c = [big.tile([128, S], bf16, tag=f"qk{c}", name=f"qk{c}")
                    for c in range(3)]
            q2scr = big.tile([128, S], bf16, tag="q2scr", name="q2scr")
            v2_sb = [big.tile([128, 512], bf16, tag=f"v2_{s}", name=f"v2_{s}")
                     for s in range(NKB // 2)]
            oTa_q = [big.tile([128, QB], bf16, tag=f"oTa{qb}",
                              name=f"oTa{qb}") for qb in range(NQB)]
            oTb_q = [big.tile([64, QB], bf16, tag=f"oTb{qb}",
                              name=f"oTb{qb}") for qb in range(NQB)]

            # ============================ phase 1: projections + RoPE
            with (
                tc.tile_pool(name="psq", bufs=2, space="PSUM") as psq_pool,
                tc.tile_pool(name="psrot", bufs=1, space="PSUM") as rot_pool,
                tc.tile_pool(name="psv", bufs=2, space="PSUM") as psv_pool,
                tc.tile_pool(name="ropetmp", bufs=2) as rtmp,
            ):
                def emit_qk_chunk(ch, half, pools=None):
                    ps_pool, r_pool, t_pool, ps_tag, t_tag = pools or (
                        psq_pool, rot_pool, rtmp, "psq", None)
                    c0 = half * 1024
                    ps = ps_pool.tile([128, 1024], f32, tag=ps_tag)
                    for n in range(4):
                        for e in range(EK):
                            nc.tensor.matmul(
                                ps[:, n * 256:(n + 1) * 256],
                                lhsT=wqk_t[e][:, ch * 128:(ch + 1) * 128],
                                rhs=xt[e][:, c0 + n * 256:c0 + (n + 1) * 256],
                                start=(e == 0), stop=(e == EK - 1),
                            )
                    # biased q -> SBUF bf16 via ScalarE (bias per partition)
                    q_sb = t_pool.tile([128, 1024], bf16,
                                       tag=t_tag or "qsb")
                    nc.scalar.activation(q_sb, ps, Act.Identity,
                                         bias=biases_t[:, ch:ch + 1])
                    # qc = q_sb * cos   (all-bf16: 2x DVE mode)
                    qc = t_pool.tile([128, 1024], bf16, tag=t_tag or "qc")
                    nc.vector.tensor_mul(qc, q_sb, cos_t[:, c0:c0 + 1024])
                    # qrot = P2 @ q_sb ; qs = qrot * sin ; out = qc + qs
                    rot = r_pool.tile([128, 1024], f32,
                                      tag="rot" if pools is None else ps_tag)
                    for n in range(2):
                        nc.tensor.matmul(
                            rot[:, n * 512:(n + 1) * 512], lhsT=p2_t,
                            rhs=q_sb[:, n * 512:(n + 1) * 512],
                            start=True, stop=True)
                    qs = t_pool.tile([128, 1024], bf16, tag=t_tag or "qs")
                    nc.vector.tensor_mul(qs, rot, sin_t[:, c0:c0 + 1024])
                    nc.vector.tensor_add(qk_c[ch][:, c0:c0 + 1024], qc, qs)
                    if ch == 2:
                        nc.vector.tensor_copy(
                            q2scr[64:128, c0:c0 + 1024],
                            qk_c[2][0:64, c0:c0 + 1024])

                def emit_v_block(s0):
                    # two sblocks share one PSUM bank; one copy out
                    ps = psv_pool.tile([128, 512], f32, tag="psv")
                    for i in range(2):
                        s = s0 + i
                        c = i * 256
                        for e in range(EK):
                            nc.tensor.matmul(
                                ps[:, c:c + 256],
                                lhsT=xt[e][:, s * 128:(s + 1) * 128],
                                rhs=wv_t[e], start=(e == 0), stop=False)
                        nc.tensor.matmul(
                            ps[:, c:c + 256],
                            lhsT=ones_row[:, s * 128:(s + 1) * 128],
                            rhs=wv7, start=False, stop=True)
                    nc.vector.tensor_copy(v2_sb[s0 // 2], ps)

                # half 0 of all 3 chunks first so attention and the
                # second x half DMA overlap phase 1's tail.
                chunks = [(ch, half) for half in range(2) for ch in range(3)]
                vs = iter(range(0, NKB, 2))
                for ch, half in chunks:
                    emit_qk_chunk(ch, half)
                    s = next(vs, None)
                    if s is not None:
                        emit_v_block(s)
                for s in vs:
                    emit_v_block(s)

            # ============================ phase 2+3: attention + y proj
            def v_lhsT(s, h):
                # head values cols [65h..65h+63] + ones col at 65h+64
                return v2_sb[s // 2][:, (s % 2) * 256 + 65 * h:
                                     (s % 2) * 256 + 65 * h + 65]

            # per-head (qT tile, kT tile, partition row for both)
            heads = ((qk_c[0], qk_c[1], 0),
                     (qk_c[0], qk_c[1], 64),
                     (q2scr, qk_c[2], 64))

            with (
                tc.tile_pool(name="ps_s", bufs=2, space="PSUM") as s_pool,
                tc.tile_pool(name="ps_ov", bufs=2, space="PSUM") as ov_pool,
                tc.tile_pool(name="ps_y", bufs=2, space="PSUM") as y_pool,
                tc.tile_pool(name="pt", bufs=6) as pt_pool,
                tc.tile_pool(name="eps", bufs=2) as ep_pool,
                tc.tile_pool(name="ysb", bufs=2) as ysb_pool,
            ):
                def emit_den(qb, h, ov):
                    # normalize, inside the NEXT unit: 1/den = exp(-ln den)
                    # on ScalarE (same activation table as the score exps),
                    # partition-broadcast via a DRAM roundtrip (stride-0
                    # read), multiply on DVE
                    import concourse.bass as _b
                    dl = ep_pool.tile([1, 512], f32, tag="dl")
                    nc.scalar.activation(dl, ov[64:65, :], Act.Ln)
                    rec_sb = ep_pool.tile([1, 512], f32, tag="den")
                    nc.scalar.activation(rec_sb, dl, Act.Exp, scale=-1.0)
                    ei = 3 * qb + h
                    nc.sync.dma_start(out=dend[ei:ei + 1, :], in_=rec_sb)
                    recb = ep_pool.tile([64, 512], f32, tag="denb")
                    dsrc = dend[ei:ei + 1, :]
                    nc.sync.dma_start(
                        out=recb,
                        in_=_b.AP(tensor=dsrc.tensor, offset=dsrc.offset,
                                  ap=[[0, 64]] + list(dsrc.ap[1:])))
                    if h < 2:
                        dst = oTa_q[qb][64 * h:64 * h + 64, :]
                    else:
                        dst = oTb_q[qb]
                    nc.vector.tensor_mul(dst, ov[0:64, :], recb)

                def emit_y(qb, mi, c0):
                    # one half-row-block piece: [128 queries, 384 cols]
                    m = 4 * qb + mi
                    yp = y_pool.tile([128, 384], f32, tag="yp")
                    nc.tensor.matmul(
                        yp,
                        lhsT=oTa_q[qb][:, mi * 128:(mi + 1) * 128],
                        rhs=wp0[:, c0:c0 + 384],
                        start=True, stop=False)
                    nc.tensor.matmul(
                        yp,
                        lhsT=oTb_q[qb][:, mi * 128:(mi + 1) * 128],
                        rhs=wp1[:, c0:c0 + 384],
                        start=False, stop=True)
                    y_sb = ysb_pool.tile([128, 384], bf16, tag="ysb")
                    nc.vector.tensor_copy(y_sb, yp)
                    nc.sync.dma_start(
                        out=y[m * 128:(m + 1) * 128, c0:c0 + 384], in_=y_sb)

                # Software pipeline: each head's den-chain (DVE/Pool only)
                # is emitted inside the NEXT head's kp loop, and each query
                # block's y projection inside the NEXT block's heads, so
                # in-order engines never head-of-line block on them.
                pend_den = None           # (qb, h, ov) awaiting den chain
                pend_y = []               # [(qb, mi)] awaiting y projection
                for qb in range(NQB):
                    for h, (qt, kt, pr) in enumerate(heads):
                        ov = ov_pool.tile([128, 512], f32, tag="ov")
                        qslice = qt[pr:pr + 64, qb * 512:(qb + 1) * 512]
                        nkb = 4 * (qb + 1)
                        # diagonal blocks first: their mask latency hides
                        # behind the past-key matmuls that follow
                        kbs = list(range(4 * qb, nkb)) + list(range(4 * qb))
                        # y half-pieces of the previous qb go into heads
                        # 1-2 at kp>=2, far enough from the den DMA
                        # roundtrip they depend on
                        y_slots = list(range(2, nkb // 2)) if h >= 1 else []
                        def emit_av(kp, pair, pt2):
                            for j, kb in enumerate(pair):
                                nc.tensor.matmul(
                                    ov[0:65, :], lhsT=v_lhsT(kb, h),
                                    rhs=pt2[:, j * 512:(j + 1) * 512],
                                    start=(2 * kp + j == 0),
                                    stop=(2 * kp + j == nkb - 1))

                        # AV runs 2 kp-steps behind scores so PE's in-order
                        # stream never blocks on the exp -> mask chain
                        inflight = []
                        for kp in range(nkb // 2):
                            pair = kbs[2 * kp:2 * kp + 2]
                            s2 = s_pool.tile([128, 1024], f32, tag="s2")
                            pt2 = pt_pool.tile([128, 1024], bf16, tag="pt2")
                            for j, kb in enumerate(pair):
                                nc.tensor.matmul(
                                    s2[:, j * 512:(j + 1) * 512],
                                    lhsT=kt[pr:pr + 64,
                                            kb * 128:(kb + 1) * 128],
                                    rhs=qslice, start=True, stop=True)
                            nc.scalar.activation(
                                pt2, s2, Act.Exp, scale=1.0 / math.sqrt(D))
                            if pair[0] >= 4 * qb:
                                # both blocks diagonal: one merged causal
                                # mask over the full tile (2D affine)
                                pt2v = pt2.rearrange(
                                    "p (j c) -> p j c", j=2)
                                nc.gpsimd.affine_select(
                                    out=pt2v, in_=pt2v,
                                    compare_op=Alu.is_ge, fill=0.0,
                                    base=qb * 512 - pair[0] * 128,
                                    channel_multiplier=-1,
                                    pattern=[[-128, 2], [1, 512]])
                            else:
                                for j, kb in enumerate(pair):
                                    if kb >= 4 * qb:  # diag: causal mask
                                        nc.gpsimd.affine_select(
                                            out=pt2[:,
                                                    j * 512:(j + 1) * 512],
                                            in_=pt2[:,
                                                    j * 512:(j + 1) * 512],
                                            compare_op=Alu.is_ge, fill=0.0,
                                            base=qb * 512 - kb * 128,
                                            channel_multiplier=-1,
                                            pattern=[[1, 512]])
                            inflight.append((kp, pair, pt2))
                            if kp == 0 and pend_den is not None:
                                emit_den(*pend_den)
                                pend_den = None
                            elif kp in y_slots and pend_y:
                                # spread the remaining pieces over the
                                # remaining slots of this qb
                                rem_slots = len(y_slots) - y_slots.index(kp) \
                                    + (len(y_slots) if h == 1 else 0)
                                take = -(-len(pend_y) // max(rem_slots, 1))
                                for _ in range(take):
                                    if pend_y:
                                        emit_y(*pend_y.pop(0))
                            if len(inflight) > 2:
                                emit_av(*inflight.pop(0))
                        for item in inflight:
                            emit_av(*item)
                        if pend_den is not None:   # corner safety
                            emit_den(*pend_den)
                        pend_den = (qb, h, ov)
                    pend_y = [(qb, mi, c0) for mi in range(4)
                              for c0 in (0, 384)]
                # drain the tail: last head's den chain + last qb's y
                emit_den(*pend_den)
                for piece in pend_y:
                    emit_y(*piece)

    _split_waits(nc)
    return nc


# ---------------------------------------------------------------- runner
class SpmdRunner:
    """Runs a Bass module on the first `n_cores` jax devices via the axon
    PJRT path (mirrors concourse.bass2jax.run_bass_via_pjrt, minus donation
    so the jitted callable is re-invocable for timing)."""

    def __init__(self, nc, n_cores=N_CORES):
        import jax
        import numpy as _np
        from jax.sharding import Mesh, PartitionSpec
        from jax.experimental.shard_map import shard_map
        import concourse.mybir as mybir
        from concourse.bass2jax import _bass_exec_p, install_neuronx_cc_hook

        install_neuronx_cc_hook()
        self.jax = jax
        self.n_cores = n_cores
        in_names, out_names, out_avals, zero_outs = [], [], [], []
        for alloc in nc.m.functions[0].allocations:
            if not isinstance(alloc, mybir.MemoryLocationSet):
                continue
            name = alloc.memorylocations[0].name
            if alloc.kind == "ExternalInput":
                in_names.append(name)
            elif alloc.kind == "ExternalOutput":
                shape = tuple(alloc.tensor_shape)
                dtype = mybir.dt.np(alloc.dtype)
                out_names.append(name)
                out_avals.append(jax.core.ShapedArray(shape, dtype))
                zero_outs.append(_np.zeros(shape, dtype))
        self.in_names, self.out_names = in_names, out_names
        self.out_avals, self.zero_outs = out_avals, zero_outs
        all_names = in_names + out_names

        def _body(*args):
            return tuple(_bass_exec_p.bind(
                *args,
                out_avals=tuple(out_avals),
                in_names=tuple(all_names),
                out_names=tuple(out_names),
                lowering_input_output_aliases=(),
                sim_require_finite=False,
                sim_require_nnan=False,
                nc=nc,
            ))

        devices = jax.devices()[:n_cores]
        self.mesh = Mesh(np.asarray(devices), ("core",))
        nin = len(in_names) + len(out_names)
        self.fn = jax.jit(
            shard_map(_body, mesh=self.mesh,
                      in_specs=(PartitionSpec("core"),) * nin,
                      out_specs=(PartitionSpec("core"),) * len(out_names),
                      check_rep=False),
            keep_unused=True,
        )
        self._dev_args = None

    def prepare(self, in_maps):
        import jax
        from jax.sharding import NamedSharding, PartitionSpec
        sharding = NamedSharding(self.mesh, PartitionSpec("core"))
        concat = [
            np.concatenate([np.ascontiguousarray(m[name]) for m in in_maps],
                           axis=0)
            for name in self.in_names
        ]
        concat += [
            np.zeros((self.n_cores * z.shape[0], *z.shape[1:]), z.dtype)
            for z in self.zero_outs
        ]
        self._dev_args = [jax.device_put(a, sharding) for a in concat]

    def run(self):
        outs = self.fn(*self._dev_args)
        self.jax.block_until_ready(outs)
        return [
            {name: np.asarray(outs[i]).reshape(
                self.n_cores, *self.out_avals[i].shape)[c]
             for i, name in enumerate(self.out_names)}
            for c in range(self.n_cores)
        ]


# ---------------------------------------------------------------- host side
def _rope_tables():
    inv_freq = 1.0 / (ROPE_BASE ** (np.arange(0, D, 2, dtype=np.float64) / D))
    t = np.arange(S, dtype=np.float64)
    freqs = np.outer(t, inv_freq)                      # [S, 32]
    emb = np.concatenate([freqs, freqs], axis=-1)      # [S, 64]
    cosT = np.cos(emb).T.astype(np.float32)            # [64, S]
    sinT = np.sin(emb).T.astype(np.float32)
    return (np.vstack([cosT, cosT]), np.vstack([sinT, sinT]))  # [128, S]


def _perm_mat():
    P = np.zeros((D, D), np.float32)
    for i in range(32):
        P[i, i + 32] = -1.0
        P[i + 32, i] = 1.0
    return P


def make_in_maps(x, Wq, bq, Wk, bk, Wv, bv, Wp, bp):
    import ml_dtypes
    bf16 = ml_dtypes.bfloat16
    cos2, sin2 = _rope_tables()
    trig = np.concatenate([cos2, sin2], axis=1).astype(bf16)   # [128, 4096]
    P = _perm_mat()
    P2 = np.zeros((128, 128), np.float32)
    P2[:64, :64] = P
    P2[64:, 64:] = P
    in_maps = []
    for c in range(N_CORES):
        b, g = c // 4, c % 4
        hs = slice(192 * g, 192 * (g + 1))
        wq_s, wk_s = Wq[:, hs], Wk[:, hs]
        # chunk0 = q heads 0,1 ; chunk1 = k heads 0,1 ; chunk2 = q2|k2
        wqk_s = np.concatenate(
            [wq_s[:, 0:128], wk_s[:, 0:128],
             wq_s[:, 128:192], wk_s[:, 128:192]], axis=1)       # [768, 384]
        bqk = np.concatenate(
            [bq[hs][0:128], bk[hs][0:128],
             bq[hs][128:192], bk[hs][128:192]])                 # [384]
        biases = np.zeros((128, 3), np.float32)
        for ch in range(3):
            biases[:, ch] = bqk[128 * ch:128 * (ch + 1)]
        wv_s = np.zeros((E, 256), np.float32)
        wv7 = np.zeros(256, np.float32)
        for h in range(HPC):
            wv_s[:, 65 * h:65 * h + 64] = \
                Wv[:, 192 * g + 64 * h:192 * g + 64 * (h + 1)]
            wv7[65 * h + 64] = 1.0
        smalls = np.zeros((128, 2432), np.float32)
        smalls[:, 0:128] = P2.T
        smalls[0, 128:128 + S] = 1.0
        smalls[0, 128 + S:128 + S + 256] = wv7
        in_maps.append({
            "xT": np.ascontiguousarray(x[b].T).astype(bf16),
            "wqk": wqk_s.astype(bf16),
            "wv": wv_s.astype(bf16),
            "wp": np.ascontiguousarray(Wp[hs, :]).astype(bf16),
            "biases": biases,
            "trig": trig,
            "smalls": smalls.astype(bf16),
        })
    return in_maps


def get_runner():
    global _RUNNER
    if _RUNNER is None:
        nc = build_bass()
        _RUNNER = SpmdRunner(nc, N_CORES)
    return _RUNNER


def assemble(results, Wp, bp, bv):
    y = np.zeros((B, S, E), np.float32)
    for c in range(N_CORES):
        y[c // 4] += results[c]["y"]
    y += (bp + bv @ Wp).astype(np.float32)
    return y


def kernel(x, Wq, bq, Wk, bk, Wv, bv, Wp, bp):
    runner = get_runner()
    runner.prepare(make_in_maps(x, Wq, bq, Wk, bk, Wv, bv, Wp, bp))
    return assemble(runner.run(), Wp, bp, bv)


# revision 64
# speedup vs baseline: 1.2748x; 1.0876x over previous
"""Causal multi-head attention with RoPE on 8 Trainium2 NeuronCores (Bass/Tile).

Problem: B=2, S=2048, E=768, H=12 heads, D=64, full rotary (ROPE_DIM=D),
causal softmax, fused QKV + output projection.

Sharding: 8 cores = 2 batches x 4 head-groups (3 heads each).

v2 (bf16, ~146-153us vs the 207us fp32r baseline):
  - all matmul inputs bf16 (PSUM accumulation stays fp32); y partials
    written back as bf16 and upcast on the host,
  - q&k projections fused into 3 full 128-partition chunks (2 head-dim
    blocks each: [q0|q1], [k0|k1], [q2|k2]) so neither the PE array nor
    the RoPE vector ops ever run at M=64; q2 re-copied to partition
    rows 64:128 (4x-mode DVE copy) so head 2's score matmul sees q and
    k at the same base partition,
  - q/k bias folded into the ScalarE PSUM->SBUF copy (per-partition
    bias operand) so RoPE is two 2x-mode bf16 DVE multiplies + one add,
  - attention per (query-block, head) unit with diagonal key-blocks
    first and the AV matmuls software-pipelined 2 score-tiles behind,
    so PE's in-order stream never blocks on the exp -> causal-mask
    chain (ScalarE exp -> gpsimd affine_select -> AV),
  - softmax denominator 1/den = exp(-ln den) on ScalarE (same
    activation table as the score exps -> no table reloads), partition-
    broadcast via a DRAM-roundtrip stride-0 DMA, multiply on DVE; the
    whole chain is emitted inside the NEXT unit's kp loop and each
    query block's y projection (bf16 half-row pieces) inside the NEXT
    block's heads 1-2 at kp>=2, so in-order engines never head-of-line
    block on them,
  - x streamed in eighth-column chunks and wqk split per chunk so the
    first projection matmul starts ~4us in.
Rejected on HW measurement: K=1 ones-matmul broadcast of 1/den
(PE slow path), DVE InstReciprocal (~5x slower than modeled), moving
the half-1 projections into the attention phase (pt/s2 pool
contention), gpsimd partition_broadcast + custom-DVE ops (this
walrus can't codegen InstISA), MAX_WAITS>1 (walrus cap).
Host sums the 4 partials per batch and adds (bp + bv @ Wp) once.
"""
import math

import numpy as np

N_HEADS = 12
ROPE_BASE = 10000.0
B, S, E = 2, 2048, 768
D = 64
HPC = 3            # heads per core
N_CORES = 8
QB = 512           # query block (free dim of score tiles)
KB = 128           # key block (partition dim of score tiles)
NQB = S // QB      # 4
NKB = S // KB      # 16
EK = E // 128      # 6 contraction chunks

_RUNNER = None


# ---------------------------------------------------------------- tile patch
def _patch_tile_drain():
    """This container's walrus caps semaphore waits per instruction ("Too
    many sync wait commands").  Split the TileContext tail-drain waits
    across dedicated SP nops."""
    import concourse.tile as tile
    import concourse.mybir as mybir

    if getattr(tile.TileContext, "_drain_patched", False):
        return

    def _drain_and_barrier(self, tick_clock, wait_clock):
        nc = self.nc
        drain_inst = nc.sync.drain()
        wait_clock.add_sem_waits(
            drain_inst.ins, tile.ScopedClock({None: tick_clock.global_clock})
        )
        si = drain_inst.ins.sync_info
        waits = list(si.on_wait) if si is not None else []
        if len(waits) > 1:
            drain_inst.ins.sync_info.on_wait = waits[:1]
            for w in waits[1:]:
                n = nc.sync.nop(nofuse=True)
                n.ins.sync_info = mybir.SyncInfo(on_wait=[w], on_update=[])
        nc.all_engine_barrier()
        assert self.sems is not None
        popped = nc._tile_sem_poison_stack.pop()
        assert popped is self._sem_poison
        nc.clear_and_free_semaphores(list(self.sems.allocated().values()))
        nc.all_engine_barrier()

    tile.TileContext._drain_and_barrier = _drain_and_barrier
    tile.TileContext._drain_patched = True


MAX_WAITS = 1


def _split_waits(nc, maxw=None):
    """Move excess semaphore waits onto same-engine NoOps inserted just
    before the carrying instruction (walrus per-instruction wait cap)."""
    import concourse.mybir as mybir

    if maxw is None:
        maxw = MAX_WAITS
    k = 0
    for f in nc.m.functions:
        for bb in f.blocks:
            new = []
            for ins in bb.instructions:
                si = ins.sync_info
                if si is not None and len(si.on_wait) > maxw:
                    waits = list(si.on_wait)
                    head, tail = waits[:-maxw], waits[-maxw:]
                    for i in range(0, len(head), maxw):
                        nop = mybir.InstNoOp(
                            name=f"{ins.name}-sw{k}", ins=[], outs=[])
                        k += 1
                        nop.engine = ins.engine
                        nop.sync_info = mybir.SyncInfo(
                            on_wait=head[i:i + maxw], on_update=[])
                        new.append(nop)
                    si.on_wait = tail
                new.append(ins)
            bb.instructions = new


# ---------------------------------------------------------------- device IR
def build_bass(reps=1):
    """reps>1 wraps the whole kernel in an on-device For_i repeat loop --
    used only for timing (slope between rep counts removes dispatch
    overhead)."""
    import contextlib
    import concourse.bass as bass
    import concourse.mybir as mybir
    import concourse.tile as tile

    _patch_tile_drain()
    f32 = mybir.dt.float32
    bf16 = mybir.dt.bfloat16
    Act = mybir.ActivationFunctionType
    Alu = mybir.AluOpType

    nc = bass.Bass(enable_partition_id=False)
    xT = nc.dram_tensor("xT", [E, S], bf16, kind="ExternalInput")
    wqk = nc.dram_tensor("wqk", [E, 384], bf16, kind="ExternalInput")
    wv = nc.dram_tensor("wv", [E, 384], bf16, kind="ExternalInput")
    wp = nc.dram_tensor("wp", [HPC * D, E], bf16, kind="ExternalInput")
    biasesd = nc.dram_tensor("biases", [128, 3], f32, kind="ExternalInput")
    trigd = nc.dram_tensor("trig", [128, 2 * S], bf16, kind="ExternalInput")
    smallsd = nc.dram_tensor("smalls", [128, 2560], bf16, kind="ExternalInput")
    dend = nc.dram_tensor("dend", [12, 512], f32, kind="Internal")
    y = nc.dram_tensor("y", [S, E], bf16, kind="ExternalOutput")

    with tile.TileContext(nc) as tc:
        rep_loop = (
            tc.For_i(0, reps, 1,
                     hint_engines=(mybir.EngineType.PE, mybir.EngineType.DVE,
                                   mybir.EngineType.Activation,
                                   mybir.EngineType.Pool, mybir.EngineType.SP))
            if reps > 1 else contextlib.nullcontext()
        )
        with rep_loop, (
            tc.tile_pool(name="consts", bufs=1)
        ) as consts, tc.tile_pool(name="big", bufs=1) as big:
            # ---- constant loads.  wqk first (gates the first matmul),
            # then x in quarter-column chunks so compute starts early.
            wqk_all = consts.tile([128, EK * 384], bf16, tag="wqk_all")
            wqk_s = wqk_all.rearrange("p (a m) -> p a m", a=EK)
            wqk_d = wqk.rearrange("(a p) m -> p a m", p=128)
            nc.sync.dma_start(out=wqk_s[:, :, 0:128], in_=wqk_d[:, :, 0:128])
            wqk_t = [wqk_all[:, e * 384:(e + 1) * 384] for e in range(EK)]
            biases_t = consts.tile([128, 3], f32, tag="biases")
            nc.sync.dma_start(out=biases_t, in_=biasesd[:, :])
            xt_all = big.tile([128, EK * S], bf16, tag="xt_all")
            xt3 = xt_all.rearrange("p (a m) -> p a m", a=EK)
            xs3 = xT.rearrange("(a p) m -> p a m", p=128)

            def load_x(i8, n=1):
                nc.sync.dma_start(
                    out=xt3[:, :, i8 * 256:(i8 + n) * 256],
                    in_=xs3[:, :, i8 * 256:(i8 + n) * 256])

            load_x(0)
            load_x(1)
            nc.sync.dma_start(out=wqk_s[:, :, 128:384],
                              in_=wqk_d[:, :, 128:384])
            xt = [xt_all[:, e * S:(e + 1) * S] for e in range(EK)]
            smalls_t = consts.tile([128, 2560], bf16, tag="smalls")
            nc.sync.dma_start(out=smalls_t, in_=smallsd[:, :])
            p2_t = smalls_t[:, 0:128]
            ones_row = smalls_t[0:1, 128:128 + S]
            wv7 = smalls_t[0:1, 128 + S:128 + S + 384]
            wv_all = consts.tile([128, EK * 384], bf16, tag="wv_all")
            nc.sync.dma_start(
                out=wv_all.rearrange("p (a m) -> p a m", a=EK),
                in_=wv.rearrange("(a p) m -> p a m", p=128))
            wv_t = [wv_all[:, e * 384:(e + 1) * 384] for e in range(EK)]
            load_x(2, 2)
            trig_t = consts.tile([128, 2 * S], bf16, tag="trig")
            nc.sync.dma_start(out=trig_t, in_=trigd[:, :])
            cos_t = trig_t[:, 0:S]
            sin_t = trig_t[:, S:2 * S]
            load_x(4, 2)
            load_x(6, 2)
            wp0 = consts.tile([128, E], bf16, tag="wp0")
            nc.sync.dma_start(out=wp0, in_=wp[0:128, :])
            wp1 = consts.tile([64, E], bf16, tag="wp1")
            nc.sync.dma_start(out=wp1, in_=wp[128:192, :])

            # ---- long-lived activations: 3 fused q|k chunks, each 128
            # partitions = 2 head-dim blocks of 64.
            # chunk0 = q heads 0,1 ; chunk1 = k heads 0,1 ;
            # chunk2 = q head 2 | k head 2.  Scores need lhsT/rhs at the
            # same base partition, so q2 is re-copied to rows 64:128 of a
            # scratch tile (4x-mode DVE copy).
            qk_c = [big.tile([128, S], bf16, tag=f"qk{c}", name=f"qk{c}")
                    for c in range(3)]
            q2scr = big.tile([128, S], bf16, tag="q2scr", name="q2scr")
            v2_sb = [big.tile([128, 768], bf16, tag=f"v2_{s}", name=f"v2_{s}")
                     for s in range(NKB // 2)]
            oTa_q = [big.tile([128, QB], bf16, tag=f"oTa{qb}",
                              name=f"oTa{qb}") for qb in range(NQB)]
            oTb_q = [big.tile([64, QB], bf16, tag=f"oTb{qb}",
                              name=f"oTb{qb}") for qb in range(NQB)]

            # ============================ phase 1: projections + RoPE
            with (
                tc.tile_pool(name="psq", bufs=2, space="PSUM") as psq_pool,
                tc.tile_pool(name="psrot", bufs=1, space="PSUM") as rot_pool,
                tc.tile_pool(name="psv", bufs=2, space="PSUM") as psv_pool,
                tc.tile_pool(name="ropetmp", bufs=2) as rtmp,
            ):
                def emit_qk_chunk(ch, half, pools=None):
                    ps_pool, r_pool, t_pool, ps_tag, t_tag = pools or (
                        psq_pool, rot_pool, rtmp, "psq", None)
                    c0 = half * 1024
                    ps = ps_pool.tile([128, 1024], f32, tag=ps_tag)
                    for n in range(4):
                        for e in range(EK):
                            nc.tensor.matmul(
                                ps[:, n * 256:(n + 1) * 256],
                                lhsT=wqk_t[e][:, ch * 128:(ch + 1) * 128],
                                rhs=xt[e][:, c0 + n * 256:c0 + (n + 1) * 256],
                                start=(e == 0), stop=(e == EK - 1),
                            )
                    # biased q -> SBUF bf16 via ScalarE (bias per partition)
                    q_sb = t_pool.tile([128, 1024], bf16,
                                       tag=t_tag or "qsb")
                    nc.scalar.activation(q_sb, ps, Act.Identity,
                                         bias=biases_t[:, ch:ch + 1])
                    # qc = q_sb * cos   (all-bf16: 2x DVE mode)
                    qc = t_pool.tile([128, 1024], bf16, tag=t_tag or "qc")
                    nc.vector.tensor_mul(qc, q_sb, cos_t[:, c0:c0 + 1024])
                    # qrot = P2 @ q_sb ; qs = qrot * sin ; out = qc + qs
                    rot = r_pool.tile([128, 1024], f32,
                                      tag="rot" if pools is None else ps_tag)
                    for n in range(2):
                        nc.tensor.matmul(
                            rot[:, n * 512:(n + 1) * 512], lhsT=p2_t,
                            rhs=q_sb[:, n * 512:(n + 1) * 512],
                            start=True, stop=True)
                    qs = t_pool.tile([128, 1024], bf16, tag=t_tag or "qs")
                    nc.vector.tensor_mul(qs, rot, sin_t[:, c0:c0 + 1024])
                    nc.vector.tensor_add(qk_c[ch][:, c0:c0 + 1024], qc, qs)
                    if ch == 2:
                        nc.vector.tensor_copy(
                            q2scr[64:128, c0:c0 + 1024],
                            qk_c[2][0:64, c0:c0 + 1024])

                def emit_v_block(s):
                    ps = psv_pool.tile([128, 384], f32, tag="psv")
                    for e in range(EK):
                        nc.tensor.matmul(
                            ps, lhsT=xt[e][:, s * 128:(s + 1) * 128],
                            rhs=wv_t[e], start=(e == 0), stop=False)
                    nc.tensor.matmul(
                        ps, lhsT=ones_row[:, s * 128:(s + 1) * 128],
                        rhs=wv7, start=False, stop=True)
                    nc.vector.tensor_copy(
                        v2_sb[s // 2][:, (s % 2) * 384:(s % 2) * 384 + 384],
                        ps)

                # half 0 of all 3 chunks first so attention and the
                # second x half DMA overlap phase 1's tail.
                chunks = [(ch, half) for half in range(2) for ch in range(3)]
                vs = iter(range(NKB))
                for ch, half in chunks:
                    emit_qk_chunk(ch, half)
                    for s in (next(vs, None), next(vs, None)):
                        if s is not None:
                            emit_v_block(s)
                for s in vs:
                    emit_v_block(s)

            # ============================ phase 2+3: attention + y proj
            def v_lhsT(s, h):
                # head values cols [128h..128h+63] + ones cols at 128h+64
                # and 128h+96 (planting den at ov rows 64 AND 96, one per
                # 32-partition quadrant, for the stream_shuffle broadcast)
                return v2_sb[s // 2][:, (s % 2) * 384 + 128 * h:
                                     (s % 2) * 384 + 128 * h + 128]

            # per-head (qT tile, kT tile, partition row for both)
            heads = ((qk_c[0], qk_c[1], 0),
                     (qk_c[0], qk_c[1], 64),
                     (q2scr, qk_c[2], 64))

            with (
                tc.tile_pool(name="ps_s", bufs=2, space="PSUM") as s_pool,
                tc.tile_pool(name="ps_ov", bufs=2, space="PSUM") as ov_pool,
                tc.tile_pool(name="ps_y", bufs=2, space="PSUM") as y_pool,
                tc.tile_pool(name="pt", bufs=6) as pt_pool,
                tc.tile_pool(name="eps", bufs=2) as ep_pool,
                tc.tile_pool(name="ysb", bufs=2) as ysb_pool,
            ):
                def emit_den(qb, h, ov):
                    # normalize, inside the NEXT unit: den sits at ov rows
                    # 64 and 96 (one per 32-partition quadrant), so one
                    # DVE stream_shuffle broadcasts it to 64 partitions
                    # straight from PSUM; 1/den = exp(-ln den) on ScalarE
                    # costs the same on [64,512] as on [1,512]
                    denb = ep_pool.tile([64, 512], f32, tag="dnb")
                    nc.vector.stream_shuffle(denb, ov[64:128, :], [0] * 32)
                    dl = ep_pool.tile([64, 512], f32, tag="dl")
                    nc.scalar.activation(dl, denb, Act.Ln)
                    recb = ep_pool.tile([64, 512], f32, tag="denb")
                    nc.scalar.activation(recb, dl, Act.Exp, scale=-1.0)
                    if h < 2:
                        dst = oTa_q[qb][64 * h:64 * h + 64, :]
                    else:
                        dst = oTb_q[qb]
                    nc.vector.tensor_mul(dst, ov[0:64, :], recb)

                def emit_y(qb, mi, c0):
                    # one half-row-block piece: [128 queries, 384 cols]
                    m = 4 * qb + mi
                    yp = y_pool.tile([128, 384], f32, tag="yp")
                    nc.tensor.matmul(
                        yp,
                        lhsT=oTa_q[qb][:, mi * 128:(mi + 1) * 128],
                        rhs=wp0[:, c0:c0 + 384],
                        start=True, stop=False)
                    nc.tensor.matmul(
                        yp,
                        lhsT=oTb_q[qb][:, mi * 128:(mi + 1) * 128],
                        rhs=wp1[:, c0:c0 + 384],
                        start=False, stop=True)
                    y_sb = ysb_pool.tile([128, 384], bf16, tag="ysb")
                    nc.vector.tensor_copy(y_sb, yp)
                    nc.sync.dma_start(
                        out=y[m * 128:(m + 1) * 128, c0:c0 + 384], in_=y_sb)

                # Software pipeline: each head's den-chain (DVE/Pool only)
                # is emitted inside the NEXT head's kp loop, and each query
                # block's y projection inside the NEXT block's heads, so
                # in-order engines never head-of-line block on them.
                pend_den = None           # (qb, h, ov) awaiting den chain
                pend_y = []               # [(qb, mi)] awaiting y projection
                for qb in range(NQB):
                    for h, (qt, kt, pr) in enumerate(heads):
                        ov = ov_pool.tile([128, 512], f32, tag="ov")
                        qslice = qt[pr:pr + 64, qb * 512:(qb + 1) * 512]
                        nkb = 4 * (qb + 1)
                        # diagonal blocks first: their mask latency hides
                        # behind the past-key matmuls that follow
                        kbs = list(range(4 * qb, nkb)) + list(range(4 * qb))
                        # y half-pieces of the previous qb go into heads
                        # 1-2 at kp>=2, far enough from the den DMA
                        # roundtrip they depend on
                        y_slots = list(range(2, nkb // 2)) if h >= 1 else []
                        def emit_av(kp, pair, pt2):
                            for j, kb in enumerate(pair):
                                nc.tensor.matmul(
                                    ov[0:128, :], lhsT=v_lhsT(kb, h),
                                    rhs=pt2[:, j * 512:(j + 1) * 512],
                                    start=(2 * kp + j == 0),
                                    stop=(2 * kp + j == nkb - 1))

                        # AV runs 2 kp-steps behind scores so PE's in-order
                        # stream never blocks on the exp -> mask chain
                        inflight = []
                        for kp in range(nkb // 2):
                            pair = kbs[2 * kp:2 * kp + 2]
                            s2 = s_pool.tile([128, 1024], f32, tag="s2")
                            pt2 = pt_pool.tile([128, 1024], bf16, tag="pt2")
                            for j, kb in enumerate(pair):
                                nc.tensor.matmul(
                                    s2[:, j * 512:(j + 1) * 512],
                                    lhsT=kt[pr:pr + 64,
                                            kb * 128:(kb + 1) * 128],
                                    rhs=qslice, start=True, stop=True)
                            nc.scalar.activation(
                                pt2, s2, Act.Exp, scale=1.0 / math.sqrt(D))
                            if pair[0] >= 4 * qb:
                                # both blocks diagonal: one merged causal
                                # mask over the full tile (2D affine)
                                pt2v = pt2.rearrange(
                                    "p (j c) -> p j c", j=2)
                                nc.gpsimd.affine_select(
                                    out=pt2v, in_=pt2v,
                                    compare_op=Alu.is_ge, fill=0.0,
                                    base=qb * 512 - pair[0] * 128,
                                    channel_multiplier=-1,
                                    pattern=[[-128, 2], [1, 512]])
                            else:
                                for j, kb in enumerate(pair):
                                    if kb >= 4 * qb:  # diag: causal mask
                                        nc.gpsimd.affine_select(
                                            out=pt2[:,
                                                    j * 512:(j + 1) * 512],
                                            in_=pt2[:,
                                                    j * 512:(j + 1) * 512],
                                            compare_op=Alu.is_ge, fill=0.0,
                                            base=qb * 512 - kb * 128,
                                            channel_multiplier=-1,
                                            pattern=[[1, 512]])
                            inflight.append((kp, pair, pt2))
                            if kp == 0 and pend_den is not None:
                                emit_den(*pend_den)
                                pend_den = None
                            elif kp in y_slots and pend_y:
                                # spread the remaining pieces over the
                                # remaining slots of this qb
                                rem_slots = len(y_slots) - y_slots.index(kp) \
                                    + (len(y_slots) if h == 1 else 0)
                                take = -(-len(pend_y) // max(rem_slots, 1))
                                for _ in range(take):
                                    if pend_y:
                                        emit_y(*pend_y.pop(0))
                            if len(inflight) > 2:
                                emit_av(*inflight.pop(0))
                        for item in inflight:
                            emit_av(*item)
                        if pend_den is not None:   # corner safety
                            emit_den(*pend_den)
                        pend_den = (qb, h, ov)
                    pend_y = [(qb, mi, c0) for mi in range(4)
                              for c0 in (0, 384)]
                # drain the tail: last head's den chain + last qb's y
                emit_den(*pend_den)
                for piece in pend_y:
                    emit_y(*piece)

    _split_waits(nc)
    return nc


# ---------------------------------------------------------------- runner
class SpmdRunner:
    """Runs a Bass module on the first `n_cores` jax devices via the axon
    PJRT path (mirrors concourse.bass2jax.run_bass_via_pjrt, minus donation
    so the jitted callable is re-invocable for timing)."""

    def __init__(self, nc, n_cores=N_CORES):
        import jax
        import numpy as _np
        from jax.sharding import Mesh, PartitionSpec
        from jax.experimental.shard_map import shard_map
        import concourse.mybir as mybir
        from concourse.bass2jax import _bass_exec_p, install_neuronx_cc_hook

        install_neuronx_cc_hook()
        self.jax = jax
        self.n_cores = n_cores
        in_names, out_names, out_avals, zero_outs = [], [], [], []
        for alloc in nc.m.functions[0].allocations:
            if not isinstance(alloc, mybir.MemoryLocationSet):
                continue
            name = alloc.memorylocations[0].name
            if alloc.kind == "ExternalInput":
                in_names.append(name)
            elif alloc.kind == "ExternalOutput":
                shape = tuple(alloc.tensor_shape)
                dtype = mybir.dt.np(alloc.dtype)
                out_names.append(name)
                out_avals.append(jax.core.ShapedArray(shape, dtype))
                zero_outs.append(_np.zeros(shape, dtype))
        self.in_names, self.out_names = in_names, out_names
        self.out_avals, self.zero_outs = out_avals, zero_outs
        all_names = in_names + out_names

        def _body(*args):
            return tuple(_bass_exec_p.bind(
                *args,
                out_avals=tuple(out_avals),
                in_names=tuple(all_names),
                out_names=tuple(out_names),
                lowering_input_output_aliases=(),
                sim_require_finite=False,
                sim_require_nnan=False,
                nc=nc,
            ))

        devices = jax.devices()[:n_cores]
        self.mesh = Mesh(np.asarray(devices), ("core",))
        nin = len(in_names) + len(out_names)
        self.fn = jax.jit(
            shard_map(_body, mesh=self.mesh,
                      in_specs=(PartitionSpec("core"),) * nin,
                      out_specs=(PartitionSpec("core"),) * len(out_names),
                      check_rep=False),
            keep_unused=True,
        )
        self._dev_args = None

    def prepare(self, in_maps):
        import jax
        from jax.sharding import NamedSharding, PartitionSpec
        sharding = NamedSharding(self.mesh, PartitionSpec("core"))
        concat = [
            np.concatenate([np.ascontiguousarray(m[name]) for m in in_maps],
                           axis=0)
            for name in self.in_names
        ]
        concat += [
            np.zeros((self.n_cores * z.shape[0], *z.shape[1:]), z.dtype)
            for z in self.zero_outs
        ]
        self._dev_args = [jax.device_put(a, sharding) for a in concat]

    def run(self):
        outs = self.fn(*self._dev_args)
        self.jax.block_until_ready(outs)
        return [
            {name: np.asarray(outs[i]).reshape(
                self.n_cores, *self.out_avals[i].shape)[c]
             for i, name in enumerate(self.out_names)}
            for c in range(self.n_cores)
        ]


# ---------------------------------------------------------------- host side
def _rope_tables():
    inv_freq = 1.0 / (ROPE_BASE ** (np.arange(0, D, 2, dtype=np.float64) / D))
    t = np.arange(S, dtype=np.float64)
    freqs = np.outer(t, inv_freq)                      # [S, 32]
    emb = np.concatenate([freqs, freqs], axis=-1)      # [S, 64]
    cosT = np.cos(emb).T.astype(np.float32)            # [64, S]
    sinT = np.sin(emb).T.astype(np.float32)
    return (np.vstack([cosT, cosT]), np.vstack([sinT, sinT]))  # [128, S]


def _perm_mat():
    P = np.zeros((D, D), np.float32)
    for i in range(32):
        P[i, i + 32] = -1.0
        P[i + 32, i] = 1.0
    return P


def make_in_maps(x, Wq, bq, Wk, bk, Wv, bv, Wp, bp):
    import ml_dtypes
    bf16 = ml_dtypes.bfloat16
    cos2, sin2 = _rope_tables()
    trig = np.concatenate([cos2, sin2], axis=1).astype(bf16)   # [128, 4096]
    P = _perm_mat()
    P2 = np.zeros((128, 128), np.float32)
    P2[:64, :64] = P
    P2[64:, 64:] = P
    in_maps = []
    for c in range(N_CORES):
        b, g = c // 4, c % 4
        hs = slice(192 * g, 192 * (g + 1))
        wq_s, wk_s = Wq[:, hs], Wk[:, hs]
        # chunk0 = q heads 0,1 ; chunk1 = k heads 0,1 ; chunk2 = q2|k2
        wqk_s = np.concatenate(
            [wq_s[:, 0:128], wk_s[:, 0:128],
             wq_s[:, 128:192], wk_s[:, 128:192]], axis=1)       # [768, 384]
        bqk = np.concatenate(
            [bq[hs][0:128], bk[hs][0:128],
             bq[hs][128:192], bk[hs][128:192]])                 # [384]
        biases = np.zeros((128, 3), np.float32)
        for ch in range(3):
            biases[:, ch] = bqk[128 * ch:128 * (ch + 1)]
        wv_s = np.zeros((E, 384), np.float32)
        wv7 = np.zeros(384, np.float32)
        for h in range(HPC):
            wv_s[:, 128 * h:128 * h + 64] = \
                Wv[:, 192 * g + 64 * h:192 * g + 64 * (h + 1)]
            wv7[128 * h + 64] = 1.0
            wv7[128 * h + 96] = 1.0
        smalls = np.zeros((128, 2560), np.float32)
        smalls[:, 0:128] = P2.T
        smalls[0, 128:128 + S] = 1.0
        smalls[0, 128 + S:128 + S + 384] = wv7
        in_maps.append({
            "xT": np.ascontiguousarray(x[b].T).astype(bf16),
            "wqk": wqk_s.astype(bf16),
            "wv": wv_s.astype(bf16),
            "wp": np.ascontiguousarray(Wp[hs, :]).astype(bf16),
            "biases": biases,
            "trig": trig,
            "smalls": smalls.astype(bf16),
        })
    return in_maps


def get_runner():
    global _RUNNER
    if _RUNNER is None:
        nc = build_bass()
        _RUNNER = SpmdRunner(nc, N_CORES)
    return _RUNNER


def assemble(results, Wp, bp, bv):
    y = np.zeros((B, S, E), np.float32)
    for c in range(N_CORES):
        y[c // 4] += results[c]["y"]
    y += (bp + bv @ Wp).astype(np.float32)
    return y


def kernel(x, Wq, bq, Wk, bk, Wv, bv, Wp, bp):
    runner = get_runner()
    runner.prepare(make_in_maps(x, Wq, bq, Wk, bk, Wv, bv, Wp, bp))
    return assemble(runner.run(), Wp, bp, bv)
